# revision 31
# baseline (speedup 1.0000x reference)
"""Trainium2 Bass kernel for a dense transformer encoder block (B=4, S=2048,
D=1024, H=16, MLP=4096) — fp8-e4m3 DoubleRow hybrid.

Sharding: 8 cores = 4 batch elements x 2 query-halves, no collectives. Each
core's kv sequence is host-reordered so its 1024 query tokens come first
(attention is permutation-invariant over keys); K/V are computed for the full
2048-token sequence.

Per-core dataflow is feature-major ("T" = [feature, token]); all matmuls have
contraction 128 (f32r) or 256 (fp8 DoubleRow) on partitions:
  LN1 (token-major, bn_stats) -> PE-transpose -> xnT stored fp8    [phase 1]
  Q/K/V projections: fp8 DoubleRow (weights host-scaled x32, fp8;  [phase 3]
    each instruction contracts 2 k-tiles at 0.5 cycles/row). Q/K/V psums stay
    32x-scaled in f32r; the 32x32 factor folds into the softmax exp scale.
  scores/softmax/AV unchanged from the f32r formulation: zero-padded 2-head
    packing (K=128), exp on ACT with scale 0.125/1024, fused AV+denominator
    via an augmented [V | 1 | 0] lhsT, reciprocal + DRAM-roundtrip broadcast;
    the RT multiply descales V's 32x and emits fp8 for the O-projection.
  O-projection: fp8 DoubleRow (Wo x32, descale at the psum copy)   [phase 4a]
  LN2 -> xn2T stored as fp8 hi/lo split (hi + residual)            [phase 4b]
  MLP: 3-chain error-compensated fp8 DoubleRow                     [phase 5]
    W@x ~= Wh@xh + Wh@xl + Wl@xh with hi/lo fp8 splits of both operands;
    the two cross terms ride single DoubleRow instructions whose two k-slots
    hold (Wlo, xhi) and (Whi, xlo). W1 scaled x32, W2 x64 (clears e4m3's
    subnormal floor so the lo residues survive); descales fold into the
    gelu scale and the existing psum->SBUF copies. gelu output goes through
    bf16, then hi/lo fp8 for fc2.

Numerics validated in numpy and on hardware: end-to-end relmax ~3.7e-3
(budget 2e-2). Weights are host-retiled to [tile, 128, k/2, 2, m] fp8 so each
DoubleRow lhsT slice is a contiguous block.

Cost-model (TimelineSim) span: ~687 us/core (baseline f32r kernel: ~862).
PE work drops from ~1.50M to ~1.08M cycles: QKV/O projections 393K->98K and
fc1/fc2 524K->393K (3-chain), with scores/AV (524K) left in f32r — converting
those needs 4-head score packing + fp8 softmax weights, which in turn needs
psum/ACT restructuring that the in-order engine queues punish (measured).
"""

import os
import sys

sys.path.insert(0, "/opt/trn_rl_repo")

from contextlib import ExitStack

import numpy as np

import concourse.bass as bass
import concourse.tile as tile
from concourse import bacc, bass_utils, mybir
from concourse.masks import make_identity

F32 = mybir.dt.float32
F32R = mybir.dt.float32r
BF16 = mybir.dt.bfloat16
F8 = mybir.dt.float8e4
PM = mybir.MatmulPerfMode
WS = 32.0
WS2 = 64.0
AF = mybir.ActivationFunctionType
ALU = mybir.AluOpType

B, S, D = 4, 2048, 1024
H, DH, MLP = 16, 64, 4096
P = 128
KD = D // P            # 8 partition tiles over D
FT = MLP // P          # 32 partition tiles over MLP dim
NQ = S // 2            # 1024 query tokens per core
ST = S // P            # 16 kv token tiles
QTT = NQ // P          # 8 q token tiles
QS = 512               # free-dim slice
NQS = NQ // QS         # 2
NKS = S // QS          # 4
NG = 4                 # head groups
EPS = 1e-6
DEBUG = bool(int(os.environ.get("KERNEL_DEBUG", "0")))
MLP_BF16 = bool(int(os.environ.get("KERNEL_MLP_BF16", "0")))

_CACHE = {}


def _build(ln_affine=True, mlp_bf16=True):
    nc = bacc.Bacc(None, target_bir_lowering=False, debug=False, num_devices=8)

    xkv = nc.dram_tensor("xkv", [S, D], F32, kind="ExternalInput").ap()
    # weights arrive host-tiled: [tile, p, kd, m] so each SBUF weight tile is
    # one contiguous DRAM block (4KB+ per-partition DMA chunks)
    KP = KD // 2
    Wq = nc.dram_tensor("Wq", [KD, P, KP, 2, P], F8, kind="ExternalInput").ap()
    Wk = nc.dram_tensor("Wk", [KD, P, KP, 2, P], F8, kind="ExternalInput").ap()
    Wv = nc.dram_tensor("Wv", [NG, P, KP, 2, 256], F8, kind="ExternalInput").ap()
    Wo = nc.dram_tensor("Wo", [KD, P, KP, 2, P], F8, kind="ExternalInput").ap()
    W1 = nc.dram_tensor("W1", [FT, P, KD, 2, P], F8, kind="ExternalInput").ap()
    W2 = nc.dram_tensor("W2", [KD, P, FT, 2, P], F8, kind="ExternalInput").ap()
    bq = nc.dram_tensor("bq", [D], F32, kind="ExternalInput").ap()
    bk = nc.dram_tensor("bk", [D], F32, kind="ExternalInput").ap()
    bv = nc.dram_tensor("bv", [D], F32, kind="ExternalInput").ap()
    bo = nc.dram_tensor("bo", [D], F32, kind="ExternalInput").ap()
    b1 = nc.dram_tensor("b1", [MLP], F32, kind="ExternalInput").ap()
    b2 = nc.dram_tensor("b2", [D], F32, kind="ExternalInput").ap()
    g1 = nc.dram_tensor("g1", [D], F32, kind="ExternalInput").ap()
    be1 = nc.dram_tensor("be1", [D], F32, kind="ExternalInput").ap()
    g2 = nc.dram_tensor("g2", [D], F32, kind="ExternalInput").ap()
    be2 = nc.dram_tensor("be2", [D], F32, kind="ExternalInput").ap()
    out = nc.dram_tensor("out", [NQ, D], F32, kind="ExternalOutput").ap()

    dbg = {}
    if DEBUG:
        dbg["xnkvT"] = nc.dram_tensor("d_xnkvT", [P, KD, S], F8, kind="ExternalOutput").ap()
        dbg["qt0"] = nc.dram_tensor("d_qt0", [P, 2, NQ], F32R, kind="ExternalOutput").ap()
        dbg["kt0"] = nc.dram_tensor("d_kt0", [P, 2, S], F32R, kind="ExternalOutput").ap()
        dbg["v0"] = nc.dram_tensor("d_v0", [P, ST, 2, 2, P], F32R, kind="ExternalOutput").ap()
        dbg["rt"] = nc.dram_tensor("d_rt", [P, KD, NQ], F8, kind="ExternalOutput").ap()
        dbg["e0"] = nc.dram_tensor("d_e0", [P, QS], F32R, kind="ExternalOutput").ap()
        dbg["s0"] = nc.dram_tensor("d_s0", [P, QS], F32, kind="ExternalOutput").ap()
        dbg["av0"] = nc.dram_tensor("d_av0", [65, 2, QS], F32, kind="ExternalOutput").ap()
        dbg["x2"] = nc.dram_tensor("d_x2", [P, QTT, D], F32, kind="ExternalOutput").ap()

    def bcast_ap(vec):
        # [D] dram vector -> [128, D] partition-replicated DMA source
        return bass.AP(tensor=vec.tensor, offset=vec.offset, ap=[[0, P]] + list(vec.ap))



    with tile.TileContext(nc) as tc:
        es = ExitStack()
        params = es.enter_context(tc.tile_pool(name="params", bufs=1))
        dramp = es.enter_context(tc.tile_pool(name="dram", bufs=1, space="DRAM"))
        x2d = dramp.tile([P, QTT, D], F32)

        ident_f = params.tile([P, P], F32)
        make_identity(nc, ident_f)
        ident = params.tile([P, P], F32R)
        nc.vector.tensor_copy(ident[:], ident_f[:])
        ones_f = params.tile([P, 1], F32)
        nc.vector.memset(ones_f[:, 0:1], 1.0)

        def pvec(v, n, nm):  # [n*128] -> [128, n] (dim o*128+p -> [p, o])
            t = params.tile([P, n], F32, name=nm)
            nc.sync.dma_start(t[:], v.rearrange("(o p) -> p o", p=P))
            return t

        bq_t = pvec(bq, KD, "bq_t")
        bk_t = pvec(bk, KD, "bk_t")
        bo_t = pvec(bo, KD, "bo_t")
        b2_t = pvec(b2, KD, "b2_t")
        b1_t = pvec(b1, FT, "b1_t")
        bv_rep = params.tile([P, D], F32)
        nc.gpsimd.dma_start(bv_rep[:], bcast_ap(bv))

        rt_es = ExitStack()
        rtp = rt_es.enter_context(tc.tile_pool(name="rt", bufs=1))
        RT_h = [rtp.tile([P, KD, QS], F8, name=f"RT{h}") for h in range(NQS)]

        xn_es = ExitStack()
        xnp = xn_es.enter_context(tc.tile_pool(name="xn", bufs=1))
        xn_kvT = xnp.tile([P, KD, S], F8)

        # ---- Phase 1: LN1 + transpose to feature-major ----
        with tc.tile_pool(name="p1tmp", bufs=4) as p1t, \
             tc.tile_pool(name="p1s", bufs=4) as p1s, \
             tc.tile_pool(name="ln1", bufs=1) as ln1p, \
             tc.tile_pool(name="p1ps", bufs=6, space="PSUM") as ps1:
            g1_rep = ln1p.tile([P, D], F32)
            nc.gpsimd.dma_start(g1_rep[:], bcast_ap(g1))
            be1_rep = ln1p.tile([P, D], F32)
            nc.gpsimd.dma_start(be1_rep[:], bcast_ap(be1))
            eps_t = ln1p.tile([P, 1], F32)
            nc.vector.memset(eps_t[:], EPS)

            for t in range(ST):
                x_t = p1t.tile([P, D], F32, tag="x_t")
                nc.sync.dma_start(x_t[:], xkv[t * P:(t + 1) * P, :])
                stats = p1s.tile([P, 2, 6], F32, tag="stats")
                xv = x_t[:].rearrange("p (s f) -> p s f", s=2)
                for s in range(2):
                    nc.vector.bn_stats(stats[:, s, :], xv[:, s, :])
                mv = p1s.tile([P, 2], F32, tag="mv")
                nc.vector.bn_aggr(mv[:], stats[:])
                std = p1s.tile([P, 1], F32, tag="std")
                nc.scalar.activation(std[:], mv[:, 1:2], AF.Sqrt, bias=eps_t[:])
                nc.vector.reciprocal(std[:], std[:])
                xn_t = p1t.tile([P, D], F32R, tag="xn_t")
                nc.vector.tensor_scalar(
                    xn_t[:], x_t[:], scalar1=mv[:, 0:1], scalar2=std[:],
                    op0=ALU.subtract, op1=ALU.mult)
                if ln_affine:
                    nc.vector.tensor_tensor(xn_t[:], xn_t[:], g1_rep[:], ALU.mult)
                    nc.vector.tensor_tensor(xn_t[:], xn_t[:], be1_rep[:], ALU.add)
                for j2 in range(KD // 2):
                    pst = ps1.tile([P, 2, P], F32, tag="tp")
                    for h in range(2):
                        nc.tensor.transpose(
                            pst[:, h, :].bitcast(F32R),
                            xn_t[:, (2 * j2 + h) * P:(2 * j2 + h + 1) * P], ident[:])
                    nc.vector.tensor_copy(
                        xn_kvT[:, 2 * j2:2 * j2 + 2, t * P:(t + 1) * P], pst[:])

        if DEBUG:
            nc.sync.dma_start(dbg["xnkvT"], xn_kvT[:])

        # ---- Phase 3: per-group QKV projection + attention ----
        with tc.tile_pool(name="kv", bufs=1) as kvp, \
             tc.tile_pool(name="wst", bufs=2) as wsp, \
             tc.tile_pool(name="expp", bufs=2) as expp, \
             tc.tile_pool(name="qpad", bufs=1) as qpp, \
             tc.tile_pool(name="rcbc", bufs=1) as rcp, \
             tc.tile_pool(name="aps", bufs=1, space="PSUM") as aps:

            zsc = qpp.tile([P, QS], F32)
            nc.vector.memset(zsc[:], 0.0)
            qpadA = [qpp.tile([P, QS], F32R, name=f"qpadA{i}") for i in range(1)]
            qpadB = [qpp.tile([P, QS], F32R, name=f"qpadB{i}") for i in range(1)]
            for i in range(1):
                nc.vector.tensor_copy(qpadA[i][:], zsc[:])
                nc.vector.tensor_copy(qpadB[i][:], zsc[:])

            QT_g = kvp.tile([P, 2, NQ], F32R)
            KT_g = kvp.tile([P, 2, S], F32R)
            # per (toktile, pair, head j): [V_head(64) | 1 | 0(63)]
            V_gp = kvp.tile([P, ST, 2, 2, P], F32R)
            for t in range(ST):
                nc.vector.tensor_copy(
                    V_gp[:, t], zsc[:].rearrange("p (a b m) -> p a b m", a=2, b=2))
            one_r = qpp.tile([P, 1], F32R)
            nc.vector.tensor_copy(one_r[:], ones_f[:, 0:1])
            for t in range(ST):
                for pi in range(2):
                    for j in range(2):
                        nc.vector.tensor_copy(V_gp[:, t, pi, j, 64:65], one_r[:])
            it_count = 0

            for g in range(NG):
                for pl in range(2):   # head pairs 2g, 2g+1
                    pr = 2 * g + pl
                    wq_t = wsp.tile([P, KD // 2, 2, P], F8, tag="wq_t")
                    nc.sync.dma_start(wq_t[:], Wq[pr])
                    for q in range(NQS):
                        ps = aps.tile([P, QS], F32, tag="pp", bufs=2)
                        for kp in range(KD // 2):
                            nc.tensor.matmul(
                                ps[:], wq_t[:, kp, :, :],
                                xn_kvT[:, 2 * kp:2 * kp + 2, q * QS:(q + 1) * QS],
                                start=(kp == 0), stop=(kp == KD // 2 - 1),
                                perf_mode=PM.DoubleRow, skip_group_check=True)
                        nc.vector.tensor_scalar_add(
                            QT_g[:, pl, q * QS:(q + 1) * QS], ps[:], bq_t[:, pr:pr + 1])
                    wk_t = wsp.tile([P, KD // 2, 2, P], F8, tag="wk_t")
                    nc.sync.dma_start(wk_t[:], Wk[pr])
                    for q in range(NKS):
                        ps = aps.tile([P, QS], F32, tag="pp", bufs=2)
                        for kp in range(KD // 2):
                            nc.tensor.matmul(
                                ps[:], wk_t[:, kp, :, :],
                                xn_kvT[:, 2 * kp:2 * kp + 2, q * QS:(q + 1) * QS],
                                start=(kp == 0), stop=(kp == KD // 2 - 1),
                                perf_mode=PM.DoubleRow, skip_group_check=True)
                        nc.vector.tensor_scalar_add(
                            KT_g[:, pl, q * QS:(q + 1) * QS], ps[:], bk_t[:, pr:pr + 1])
                wv_t = wsp.tile([P, KD // 2, 2, 256], F8, tag="wv_t", bufs=1)
                nc.sync.dma_start(wv_t[:], Wv[g])
                for t in range(ST):
                    ps = aps.tile([P, QS], F32, tag="pp", bufs=2)
                    for kp in range(KD // 2):
                        nc.tensor.matmul(
                            ps[:, 0:256],
                            xn_kvT[:, 2 * kp:2 * kp + 2, t * P:(t + 1) * P],
                            wv_t[:, kp, :, :],
                            start=(kp == 0), stop=(kp == KD // 2 - 1),
                            perf_mode=PM.DoubleRow, skip_group_check=True)
                    for pi in range(2):
                        nc.vector.tensor_tensor(
                            V_gp[:, t, pi, :, 0:64],
                            ps[:, pi * 128:(pi + 1) * 128].rearrange("p (j m) -> p j m", j=2),
                            bv_rep[:, g * 256 + pi * 128:g * 256 + (pi + 1) * 128].rearrange(
                                "p (j m) -> p j m", j=2), ALU.add)

                if DEBUG and g == 0:
                    nc.sync.dma_start(dbg["kt0"], KT_g[:])
                    nc.sync.dma_start(dbg["v0"], V_gp[:])
                    nc.sync.dma_start(dbg["qt0"], QT_g[:])

                for q in range(NQS):
                    for pl in range(2):
                        pr = 2 * g + pl
                        i = it_count % 1
                        it_count += 1
                        qsl = slice(q * QS, (q + 1) * QS)
                        nc.vector.tensor_copy(qpadA[i][0:64, :], QT_g[0:64, pl, qsl])
                        nc.vector.tensor_copy(qpadB[i][64:128, :], QT_g[64:128, pl, qsl])
                        av1 = aps.tile([P, QS], F32, tag="av1")
                        av2 = aps.tile([P, QS], F32, tag="av2")
                        for kt in range(ST):
                            ktsl = slice(kt * P, (kt + 1) * P)
                            sAB = aps.tile([P, 2, QS], F32, tag="sAB", bufs=2)
                            nc.tensor.matmul(sAB[:, 0, :], KT_g[:, pl, ktsl], qpadA[i][:],
                                             start=True, stop=True)
                            nc.tensor.matmul(sAB[:, 1, :], KT_g[:, pl, ktsl], qpadB[i][:],
                                             start=True, stop=True)
                            eAB = expp.tile([P, 2, QS], F32R, tag="eAB")
                            nc.scalar.activation(eAB[:], sAB[:], AF.Exp, scale=0.125 / (WS * WS))
                            eA = eAB[:, 0, :]
                            eB = eAB[:, 1, :]
                            if DEBUG and g == 0 and q == 0 and pl == 0 and kt == 0:
                                nc.sync.dma_start(dbg["e0"], eA)
                                s0c = rcp.tile([P, QS], F32, tag="s0c")
                                nc.vector.tensor_copy(s0c[:], sAB[:, 0, :])
                                nc.sync.dma_start(dbg["s0"], s0c[:])
                            st, sp = (kt == 0), (kt == ST - 1)
                            nc.tensor.matmul(av1[:], V_gp[:, kt, pl, 0, :], eA,
                                             start=st, stop=sp, skip_group_check=True)
                            nc.tensor.matmul(av2[:], V_gp[:, kt, pl, 1, :], eB,
                                             start=st, stop=sp, skip_group_check=True)
                        # free the av psums fast: copy to SBUF, divide from there
                        avc = rcp.tile([65, 2, QS], F32, tag="avc")
                        nc.vector.tensor_copy(avc[0:65, 0, :], av1[0:65, :])
                        nc.vector.tensor_copy(avc[0:65, 1, :], av2[0:65, :])
                        nc.vector.reciprocal(avc[64:65, 0, :], avc[64:65, 0, :])
                        nc.vector.reciprocal(avc[64:65, 1, :], avc[64:65, 1, :])
                        rcd = dramp.tile([2, QS], F32, tag="rcd", bufs=2)
                        nc.sync.dma_start(rcd[0:1, :], avc[64:65, 0, :])
                        nc.sync.dma_start(rcd[1:2, :], avc[64:65, 1, :])
                        bcA = rcp.tile([64, QS], F32, tag="bcA")
                        bcB = rcp.tile([64, QS], F32, tag="bcB")

                        def _b64(row_ap):
                            return bass.AP(tensor=row_ap.tensor, offset=row_ap.offset,
                                           ap=[[0, 64]] + list(row_ap.ap)[1:])

                        nc.sync.dma_start(bcA[:], _b64(rcd[0:1, :]))
                        nc.sync.dma_start(bcB[:], _b64(rcd[1:2, :]))
                        if DEBUG and g == 0 and q == 0 and pl == 0:
                            nc.sync.dma_start(dbg["av0"], avc[:])
                        nc.vector.scalar_tensor_tensor(
                            RT_h[q][0:64, pr, :], avc[0:64, 0, :], 1.0 / WS,
                            bcA[:], ALU.mult, ALU.mult)
                        stB = rcp.tile([64, QS], F8, tag="stB")
                        nc.vector.scalar_tensor_tensor(
                            stB[:], avc[0:64, 1, :], 1.0 / WS, bcB[:],
                            ALU.mult, ALU.mult)
                        nc.sync.dma_start(RT_h[q][64:128, pr, :], stB[:])

        xn_es.close()

        if DEBUG:
            for h in range(NQS):
                nc.sync.dma_start(
                    dbg["rt"].rearrange("p k (h w) -> p k h w", h=NQS)[:, :, h, :], RT_h[h][:])

        # ---- Phase 4a: O-projection + residual -> x2 (DRAM) ----
        with tc.tile_pool(name="p4tmp", bufs=2) as p4t, \
             tc.tile_pool(name="p4ps", bufs=2, space="PSUM") as ps4, \
             tc.tile_pool(name="p4tps", bufs=6, space="PSUM") as ps4t:
            for q in range(NQS):
                attnT = p4t.tile([P, KD, QS], F32R, tag="attnT")
                for mt in range(KD):
                    wo_t = p4t.tile([P, KD // 2, 2, P], F8, tag="wo_t")
                    nc.sync.dma_start(wo_t[:], Wo[mt])
                    ps = ps4.tile([P, QS], F32, tag="pp")
                    for kp in range(KD // 2):
                        nc.tensor.matmul(
                            ps[:], wo_t[:, kp, :, :],
                            RT_h[q][:, 2 * kp:2 * kp + 2, :],
                            start=(kp == 0), stop=(kp == KD // 2 - 1),
                            perf_mode=PM.DoubleRow, skip_group_check=True)
                    nc.vector.tensor_scalar(
                        attnT[:, mt, :], ps[:], scalar1=1.0 / WS,
                        scalar2=bo_t[:, mt:mt + 1], op0=ALU.mult, op1=ALU.add)
                for j in range(QS // P):
                    tt = q * (QS // P) + j
                    xr_t = p4t.tile([P, D], F32, tag="xr_t")
                    nc.sync.dma_start(xr_t[:], xkv[tt * P:(tt + 1) * P, :])
                    x2_t = p4t.tile([P, D], F32, tag="x2_t")
                    for m2 in range(KD // 2):
                        pst = ps4t.tile([P, 2, P], F32, tag="tp")
                        for h in range(2):
                            nc.tensor.transpose(
                                pst[:, h, :].bitcast(F32R),
                                attnT[:, 2 * m2 + h, j * P:(j + 1) * P], ident[:])
                        nc.vector.tensor_tensor(
                            x2_t[:, 2 * m2 * P:(2 * m2 + 2) * P],
                            pst[:].rearrange("p a m -> p (a m)"),
                            xr_t[:, 2 * m2 * P:(2 * m2 + 2) * P], ALU.add)
                    nc.sync.dma_start(x2d[:, tt, :], x2_t[:])
                    if DEBUG:
                        nc.sync.dma_start(dbg["x2"][:, tt, :], x2_t[:])
        rt_es.close()

        # ---- Phase 4b: LN2 -> xn2T ----
        xn2_es = ExitStack()
        xn2p = xn2_es.enter_context(tc.tile_pool(name="xn2", bufs=1))
        xn2T_h = [xn2p.tile([P, KD, 2, QS], F8, name=f"xn2T{h}") for h in range(NQS)]
        with tc.tile_pool(name="p4btmp", bufs=4) as p4bt, \
             tc.tile_pool(name="p4bs", bufs=4) as p4bs, \
             tc.tile_pool(name="ln2", bufs=1) as ln2p, \
             tc.tile_pool(name="p4bps", bufs=6, space="PSUM") as ps4b:
            g2_rep = ln2p.tile([P, D], F32)
            nc.gpsimd.dma_start(g2_rep[:], bcast_ap(g2))
            be2_rep = ln2p.tile([P, D], F32)
            nc.gpsimd.dma_start(be2_rep[:], bcast_ap(be2))
            eps2_t = ln2p.tile([P, 1], F32)
            nc.vector.memset(eps2_t[:], EPS)

            for tt in range(QTT):
                x2_t = p4bt.tile([P, D], F32, tag="x2_t")
                nc.sync.dma_start(x2_t[:], x2d[:, tt, :])
                stats = p4bs.tile([P, 2, 6], F32, tag="stats2")
                xv = x2_t[:].rearrange("p (s f) -> p s f", s=2)
                for s in range(2):
                    nc.vector.bn_stats(stats[:, s, :], xv[:, s, :])
                mv = p4bs.tile([P, 2], F32, tag="mv2")
                nc.vector.bn_aggr(mv[:], stats[:])
                std = p4bs.tile([P, 1], F32, tag="std2")
                nc.scalar.activation(std[:], mv[:, 1:2], AF.Sqrt, bias=eps2_t[:])
                nc.vector.reciprocal(std[:], std[:])
                xn2_t = p4bt.tile([P, D], F32R, tag="xn2_t")
                nc.vector.tensor_scalar(
                    xn2_t[:], x2_t[:], scalar1=mv[:, 0:1], scalar2=std[:],
                    op0=ALU.subtract, op1=ALU.mult)
                if ln_affine:
                    nc.vector.tensor_tensor(xn2_t[:], xn2_t[:], g2_rep[:], ALU.mult)
                    nc.vector.tensor_tensor(xn2_t[:], xn2_t[:], be2_rep[:], ALU.add)
                hs_i, loc = tt // (QS // P), (tt % (QS // P)) * P
                for j2 in range(KD // 2):
                    pst = ps4b.tile([P, 2, P], F32, tag="tp")
                    for h in range(2):
                        nc.tensor.transpose(
                            pst[:, h, :].bitcast(F32R),
                            xn2_t[:, (2 * j2 + h) * P:(2 * j2 + h + 1) * P], ident[:])
                    hi = xn2T_h[hs_i][:, 2 * j2:2 * j2 + 2, 0, loc:loc + P]
                    nc.vector.tensor_copy(hi, pst[:])
                    nc.vector.scalar_tensor_tensor(
                        xn2T_h[hs_i][:, 2 * j2:2 * j2 + 2, 1, loc:loc + P],
                        pst[:], 1.0, hi, ALU.mult, ALU.subtract)

        # ---- Phase 5: MLP (h1 in bf16, single full-width token pass) ----
        with tc.tile_pool(name="p5tmp", bufs=3) as p5t, \
             tc.tile_pool(name="h1", bufs=1) as h1p, \
             tc.tile_pool(name="w2st", bufs=2) as w2p, \
             tc.tile_pool(name="p5ps", bufs=2, space="PSUM") as ps5, \
             tc.tile_pool(name="p5tps", bufs=4, space="PSUM") as ps5t:
            n_hslice = NQS
            HW_ = NQ // n_hslice
            out_acc = h1p.tile([P, QTT, D], F32)
            for hs in range(n_hslice):
                h1T = h1p.tile([P, FT, 2, HW_], F8, tag="h1T")
                for ft in range(FT):
                    w1_t = p5t.tile([P, KD, 2, P], F8, tag="w1_t")
                    nc.sync.dma_start(w1_t[:], W1[ft])
                    for sl in range(HW_ // QS):
                        gsl = (hs * HW_ + sl * QS) // QS
                        ps = ps5.tile([P, QS], F32, tag="pp")
                        for kp in range(KD // 2):
                            nc.tensor.matmul(
                                ps[:], w1_t[:, 2 * kp:2 * kp + 2, 1, :],
                                xn2T_h[gsl][:, 2 * kp:2 * kp + 2, 0, :],
                                start=(kp == 0), stop=False,
                                perf_mode=PM.DoubleRow, skip_group_check=True)
                        for k in range(KD):
                            nc.tensor.matmul(
                                ps[:], w1_t[:, k, :, :], xn2T_h[gsl][:, k, :, :],
                                start=False, stop=(k == KD - 1),
                                perf_mode=PM.DoubleRow, skip_group_check=True)
                        h1b = p5t.tile([P, QS], BF16, tag="h1b", bufs=2)
                        nc.scalar.activation(h1b[:], ps[:], AF.Gelu,
                                             bias=b1_t[:, ft:ft + 1], scale=1.0 / WS)
                        hi = h1T[:, ft, 0, sl * QS:(sl + 1) * QS]
                        nc.gpsimd.tensor_copy(hi, h1b[:])
                        nc.vector.scalar_tensor_tensor(
                            h1T[:, ft, 1, sl * QS:(sl + 1) * QS], h1b[:],
                            1.0, hi, ALU.mult, ALU.subtract)
                for mt in range(KD):
                    w2_t = w2p.tile([P, FT, 2, P], F8, tag="w2_t")
                    nc.sync.dma_start(w2_t[:], W2[mt])
                    for sl in range(HW_ // QS):
                        ssl_loc = slice(sl * QS, (sl + 1) * QS)
                        ps = ps5.tile([P, QS], F32, tag="pp")
                        for fp in range(FT // 2):
                            nc.tensor.matmul(
                                ps[:], w2_t[:, 2 * fp:2 * fp + 2, 1, :],
                                h1T[:, 2 * fp:2 * fp + 2, 0, ssl_loc],
                                start=(fp == 0), stop=False,
                                perf_mode=PM.DoubleRow, skip_group_check=True)
                        for k in range(FT):
                            nc.tensor.matmul(
                                ps[:], w2_t[:, k, :, :], h1T[:, k, :, ssl_loc],
                                start=False, stop=(k == FT - 1),
                                perf_mode=PM.DoubleRow, skip_group_check=True)
                        outT = p5t.tile([P, QS], F32R, tag="outT", bufs=2)
                        nc.vector.tensor_scalar(
                            outT[:], ps[:], scalar1=1.0 / WS2,
                            scalar2=b2_t[:, mt:mt + 1], op0=ALU.mult, op1=ALU.add)
                        for j in range(QS // P):
                            tt = hs * (HW_ // P) + sl * (QS // P) + j
                            pst = ps5t.tile([P, P], F32, tag="tp")
                            nc.tensor.transpose(pst[:].bitcast(F32R),
                                                outT[:, j * P:(j + 1) * P], ident[:])
                            nc.vector.tensor_copy(out_acc[:, tt, mt * P:(mt + 1) * P], pst[:])
            for tt in range(QTT):
                x2_t = p5t.tile([P, D], F32, tag="x2r_t")
                nc.sync.dma_start(x2_t[:], x2d[:, tt, :])
                ob = p5t.tile([P, D], F32, tag="ob")
                nc.vector.tensor_tensor(ob[:], out_acc[:, tt, :], x2_t[:], ALU.add)
                nc.sync.dma_start(out[tt * P:(tt + 1) * P, :], ob[:])

        xn2_es.close()
        es.close()

    nc.compile()
    return nc


def kernel(**inputs):
    inputs = {k: np.ascontiguousarray(np.asarray(v), dtype=np.float32)
              for k, v in inputs.items()}
    ln_affine = not (
        np.all(inputs["ln1_g"] == 1.0) and np.all(inputs["ln1_b"] == 0.0)
        and np.all(inputs["ln2_g"] == 1.0) and np.all(inputs["ln2_b"] == 0.0))
    key = ("nc", ln_affine, MLP_BF16)
    if key not in _CACHE:
        _CACHE[key] = _build(ln_affine=ln_affine, mlp_bf16=MLP_BF16)
    nc = _CACHE[key]

    x = inputs["x"]
    f8 = __import__("ml_dtypes").float8_e4m3

    def attn_w(W, mw, sc):
        # [Din, Dout] -> [Dout/mw, 128, KD/2, 2, mw] scaled fp8 (DoubleRow lhsT)
        Dout = W.shape[1]
        t = (W * sc).astype(np.float32).reshape(
            KD // 2, 2, P, Dout // mw, mw).transpose(3, 2, 0, 1, 4)
        return np.ascontiguousarray(t).astype(f8)

    def mlp_w(W, sc, nk):
        # [Din, Dout] -> [Dout/128, 128, nk, 2, 128]; slot0=lo, slot1=hi
        Din, Dout = W.shape
        Ws = (W * sc).astype(np.float32)
        hi = Ws.astype(f8)
        lo = (Ws - hi.astype(np.float32)).astype(f8)

        def tile4(A):
            return A.reshape(nk, P, Dout // P, P).transpose(2, 1, 0, 3)

        outw = np.empty((Dout // P, P, nk, 2, P), dtype=f8)
        outw[:, :, :, 0, :] = tile4(lo)
        outw[:, :, :, 1, :] = tile4(hi)
        return np.ascontiguousarray(outw)

    shared = {
        "Wq": attn_w(inputs["Wq"], P, WS), "Wk": attn_w(inputs["Wk"], P, WS),
        "Wv": attn_w(inputs["Wv"], 256, WS), "Wo": attn_w(inputs["Wo"], P, WS),
        "W1": mlp_w(inputs["W1"], WS, KD),
        "W2": mlp_w(inputs["W2"], WS2, FT),
        "bq": inputs["bq"] * np.float32(WS), "bk": inputs["bk"] * np.float32(WS),
        "bv": inputs["bv"] * np.float32(WS), "bo": inputs["bo"],
        "b1": inputs["b1"], "b2": inputs["b2"],
        "g1": inputs["ln1_g"], "be1": inputs["ln1_b"],
        "g2": inputs["ln2_g"], "be2": inputs["ln2_b"],
    }
    in_maps = []
    for c in range(8):
        b, half = c // 2, c % 2
        m = dict(shared)
        # query half first; attention is permutation-invariant over kv order
        m["xkv"] = np.ascontiguousarray(
            np.concatenate([x[b, half * NQ:(half + 1) * NQ, :],
                            x[b, (1 - half) * NQ:(2 - half) * NQ, :]], axis=0))
        in_maps.append(m)

    trace = bool(int(os.environ.get("KERNEL_TRACE", "0")))
    kw = {}
    if trace:
        kw = dict(trace=True, tmpdir=os.environ.get("KERNEL_TRACE_DIR") or None)
    res = bass_utils.run_bass_kernel_spmd(nc, in_maps, core_ids=list(range(8)), **kw)
    _CACHE["last_results"] = res
    _CACHE["nc"] = nc
    _CACHE["last_in_maps"] = in_maps

    outa = np.empty((B, S, D), dtype=np.float32)
    for c in range(8):
        b, half = c // 2, c % 2
        outa[b, half * NQ:(half + 1) * NQ, :] = res.results[c]["out"]
    return outa



# revision 38
# speedup vs baseline: 1.0380x; 1.0380x over previous
"""Trainium2 Bass kernel for a dense transformer encoder block (B=4, S=2048,
D=1024, H=16, MLP=4096) — fp8-e4m3 DoubleRow hybrid.

Sharding: 8 cores = 4 batch elements x 2 query-halves, no collectives. Each
core's kv sequence is host-reordered so its 1024 query tokens come first
(attention is permutation-invariant over keys); K/V are computed for the full
2048-token sequence.

Per-core dataflow is feature-major ("T" = [feature, token]); all matmuls have
contraction 128 (f32r) or 256 (fp8 DoubleRow) on partitions:
  LN1 (token-major, bn_stats) -> PE-transpose -> xnT stored fp8    [phase 1]
  Q/K/V projections: fp8 DoubleRow (weights host-scaled x32, fp8;  [phase 3]
    each instruction contracts 2 k-tiles at 0.5 cycles/row). Q/K/V psums stay
    32x-scaled in f32r; the 32x32 factor folds into the softmax exp scale.
  scores/softmax/AV unchanged from the f32r formulation: zero-padded 2-head
    packing (K=128), exp on ACT with scale 0.125/1024, fused AV+denominator
    via an augmented [V | 1 | 0] lhsT, reciprocal + DRAM-roundtrip broadcast;
    the RT multiply descales V's 32x and emits fp8 for the O-projection.
  O-projection: fp8 DoubleRow (Wo x32, descale at the psum copy)   [phase 4a]
  LN2 -> xn2T stored as fp8 hi/lo split (hi + residual)            [phase 4b]
  MLP: 3-chain error-compensated fp8 DoubleRow                     [phase 5]
    W@x ~= Wh@xh + Wh@xl + Wl@xh with hi/lo fp8 splits of both operands;
    the two cross terms ride single DoubleRow instructions whose two k-slots
    hold (Wlo, xhi) and (Whi, xlo). W1 scaled x32, W2 x64 (clears e4m3's
    subnormal floor so the lo residues survive); descales fold into the
    gelu scale and the existing psum->SBUF copies. gelu output goes through
    bf16, then hi/lo fp8 for fc2.

Numerics validated in numpy and on hardware: end-to-end relmax ~3.7e-3
(budget 2e-2). Weights are host-retiled to [tile, 128, k/2, 2, m] fp8 so each
DoubleRow lhsT slice is a contiguous block.

Cost-model (TimelineSim) span: ~687 us/core (baseline f32r kernel: ~862).
PE work drops from ~1.50M to ~1.08M cycles: QKV/O projections 393K->98K and
fc1/fc2 524K->393K (3-chain), with scores/AV (524K) left in f32r — converting
those needs 4-head score packing + fp8 softmax weights, which in turn needs
psum/ACT restructuring that the in-order engine queues punish (measured).
"""

import os
import sys

sys.path.insert(0, "/opt/trn_rl_repo")

from contextlib import ExitStack

import numpy as np

import concourse.bass as bass
import concourse.tile as tile
from concourse import bacc, bass_utils, mybir
from concourse.masks import make_identity

F32 = mybir.dt.float32
F32R = mybir.dt.float32r
BF16 = mybir.dt.bfloat16
F8 = mybir.dt.float8e4
PM = mybir.MatmulPerfMode
WS = 32.0
WS2 = 64.0
AF = mybir.ActivationFunctionType
ALU = mybir.AluOpType

B, S, D = 4, 2048, 1024
H, DH, MLP = 16, 64, 4096
P = 128
KD = D // P            # 8 partition tiles over D
FT = MLP // P          # 32 partition tiles over MLP dim
NQ = S // 2            # 1024 query tokens per core
ST = S // P            # 16 kv token tiles
QTT = NQ // P          # 8 q token tiles
QS = 512               # free-dim slice
NQS = NQ // QS         # 2
NKS = S // QS          # 4
NG = 4                 # head groups
EPS = 1e-6
DEBUG = bool(int(os.environ.get("KERNEL_DEBUG", "0")))
MLP_BF16 = bool(int(os.environ.get("KERNEL_MLP_BF16", "0")))

_CACHE = {}


def _build(ln_affine=True, mlp_bf16=True):
    nc = bacc.Bacc(None, target_bir_lowering=False, debug=False, num_devices=8)

    xkv = nc.dram_tensor("xkv", [S, D], F32, kind="ExternalInput").ap()
    # weights arrive host-tiled: [tile, p, kd, m] so each SBUF weight tile is
    # one contiguous DRAM block (4KB+ per-partition DMA chunks)
    KP = KD // 2
    Wq = nc.dram_tensor("Wq", [KD, P, KP, 2, P], F8, kind="ExternalInput").ap()
    Wk = nc.dram_tensor("Wk", [KD, P, KP, 2, P], F8, kind="ExternalInput").ap()
    Wv = nc.dram_tensor("Wv", [NG, P, KP, 2, 256], F8, kind="ExternalInput").ap()
    Wo = nc.dram_tensor("Wo", [KD, P, KP, 2, P], F8, kind="ExternalInput").ap()
    W1 = nc.dram_tensor("W1", [FT, P, KD, 2, P], F8, kind="ExternalInput").ap()
    W2 = nc.dram_tensor("W2", [KD, P, FT, 2, P], F8, kind="ExternalInput").ap()
    bq = nc.dram_tensor("bq", [D], F32, kind="ExternalInput").ap()
    bk = nc.dram_tensor("bk", [D], F32, kind="ExternalInput").ap()
    bv = nc.dram_tensor("bv", [D], F32, kind="ExternalInput").ap()
    bo = nc.dram_tensor("bo", [D], F32, kind="ExternalInput").ap()
    b1 = nc.dram_tensor("b1", [MLP], F32, kind="ExternalInput").ap()
    b2 = nc.dram_tensor("b2", [D], F32, kind="ExternalInput").ap()
    g1 = nc.dram_tensor("g1", [D], F32, kind="ExternalInput").ap()
    be1 = nc.dram_tensor("be1", [D], F32, kind="ExternalInput").ap()
    g2 = nc.dram_tensor("g2", [D], F32, kind="ExternalInput").ap()
    be2 = nc.dram_tensor("be2", [D], F32, kind="ExternalInput").ap()
    out = nc.dram_tensor("out", [NQ, D], F32, kind="ExternalOutput").ap()

    dbg = {}
    if DEBUG:
        dbg["xnkvT"] = nc.dram_tensor("d_xnkvT", [P, KD, S], F8, kind="ExternalOutput").ap()
        dbg["qt0"] = nc.dram_tensor("d_qt0", [P, 2, NQ], F32R, kind="ExternalOutput").ap()
        dbg["kt0"] = nc.dram_tensor("d_kt0", [P, 2, S], F32R, kind="ExternalOutput").ap()
        dbg["v0"] = nc.dram_tensor("d_v0", [P, ST, 2, 2, P], F32R, kind="ExternalOutput").ap()
        dbg["rt"] = nc.dram_tensor("d_rt", [P, KD, NQ], F8, kind="ExternalOutput").ap()
        dbg["e0"] = nc.dram_tensor("d_e0", [P, QS], F32R, kind="ExternalOutput").ap()
        dbg["s0"] = nc.dram_tensor("d_s0", [P, QS], F32, kind="ExternalOutput").ap()
        dbg["av0"] = nc.dram_tensor("d_av0", [65, 2, QS], F32, kind="ExternalOutput").ap()
        dbg["x2"] = nc.dram_tensor("d_x2", [P, QTT, D], F32, kind="ExternalOutput").ap()

    def bcast_ap(vec):
        # [D] dram vector -> [128, D] partition-replicated DMA source
        return bass.AP(tensor=vec.tensor, offset=vec.offset, ap=[[0, P]] + list(vec.ap))



    with tile.TileContext(nc) as tc:
        es = ExitStack()
        params = es.enter_context(tc.tile_pool(name="params", bufs=1))
        dramp = es.enter_context(tc.tile_pool(name="dram", bufs=1, space="DRAM"))
        x2sb = es.enter_context(tc.tile_pool(name="x2sb", bufs=1))
        x2d = x2sb.tile([P, QTT, D], F32)

        ident_f = params.tile([P, P], F32)
        make_identity(nc, ident_f)
        ident = params.tile([P, P], F32R)
        nc.vector.tensor_copy(ident[:], ident_f[:])
        ones_f = params.tile([P, 1], F32)
        nc.vector.memset(ones_f[:, 0:1], 1.0)

        def pvec(v, n, nm):  # [n*128] -> [128, n] (dim o*128+p -> [p, o])
            t = params.tile([P, n], F32, name=nm)
            nc.sync.dma_start(t[:], v.rearrange("(o p) -> p o", p=P))
            return t

        bq_t = pvec(bq, KD, "bq_t")
        bk_t = pvec(bk, KD, "bk_t")
        bo_t = pvec(bo, KD, "bo_t")
        b2_t = pvec(b2, KD, "b2_t")
        b1_t = pvec(b1, FT, "b1_t")
        bv_rep = params.tile([P, D], F32)
        nc.gpsimd.dma_start(bv_rep[:], bcast_ap(bv))

        rt_es = ExitStack()
        rtp = rt_es.enter_context(tc.tile_pool(name="rt", bufs=1))
        RT_h = [rtp.tile([P, KD, QS], F8, name=f"RT{h}") for h in range(NQS)]

        xn_es = ExitStack()
        xnp = xn_es.enter_context(tc.tile_pool(name="xn", bufs=1))
        xn_kvT = xnp.tile([P, KD, S], F8)

        # ---- Phase 1: LN1 + transpose to feature-major ----
        with tc.tile_pool(name="p1tmp", bufs=4) as p1t, \
             tc.tile_pool(name="p1s", bufs=4) as p1s, \
             tc.tile_pool(name="ln1", bufs=1) as ln1p, \
             tc.tile_pool(name="p1ps", bufs=6, space="PSUM") as ps1:
            g1_rep = ln1p.tile([P, D], F32)
            nc.gpsimd.dma_start(g1_rep[:], bcast_ap(g1))
            be1_rep = ln1p.tile([P, D], F32)
            nc.gpsimd.dma_start(be1_rep[:], bcast_ap(be1))
            eps_t = ln1p.tile([P, 1], F32)
            nc.vector.memset(eps_t[:], EPS)

            for t in range(ST):
                x_t = p1t.tile([P, D], F32, tag="x_t")
                nc.sync.dma_start(x_t[:], xkv[t * P:(t + 1) * P, :])
                stats = p1s.tile([P, 2, 6], F32, tag="stats")
                xv = x_t[:].rearrange("p (s f) -> p s f", s=2)
                for s in range(2):
                    nc.vector.bn_stats(stats[:, s, :], xv[:, s, :])
                mv = p1s.tile([P, 2], F32, tag="mv")
                nc.vector.bn_aggr(mv[:], stats[:])
                std = p1s.tile([P, 1], F32, tag="std")
                nc.scalar.activation(std[:], mv[:, 1:2], AF.Sqrt, bias=eps_t[:])
                nc.vector.reciprocal(std[:], std[:])
                xn_t = p1t.tile([P, D], F32R, tag="xn_t")
                nc.vector.tensor_scalar(
                    xn_t[:], x_t[:], scalar1=mv[:, 0:1], scalar2=std[:],
                    op0=ALU.subtract, op1=ALU.mult)
                if ln_affine:
                    nc.vector.tensor_tensor(xn_t[:], xn_t[:], g1_rep[:], ALU.mult)
                    nc.vector.tensor_tensor(xn_t[:], xn_t[:], be1_rep[:], ALU.add)
                for j2 in range(KD // 2):
                    pst = ps1.tile([P, 2, P], F32, tag="tp")
                    for h in range(2):
                        nc.tensor.transpose(
                            pst[:, h, :].bitcast(F32R),
                            xn_t[:, (2 * j2 + h) * P:(2 * j2 + h + 1) * P], ident[:])
                    nc.vector.tensor_copy(
                        xn_kvT[:, 2 * j2:2 * j2 + 2, t * P:(t + 1) * P], pst[:])

        if DEBUG:
            nc.sync.dma_start(dbg["xnkvT"], xn_kvT[:])

        # ---- Phase 3: per-group QKV projection + attention ----
        with tc.tile_pool(name="kv", bufs=1) as kvp, \
             tc.tile_pool(name="wst", bufs=2) as wsp, \
             tc.tile_pool(name="expp", bufs=2) as expp, \
             tc.tile_pool(name="qpad", bufs=1) as qpp, \
             tc.tile_pool(name="rcbc", bufs=1) as rcp, \
             tc.tile_pool(name="aps", bufs=1, space="PSUM") as aps:

            zsc = qpp.tile([P, QS], F32)
            nc.vector.memset(zsc[:], 0.0)
            qpadA = [qpp.tile([P, QS], F32R, name=f"qpadA{i}") for i in range(1)]
            qpadB = [qpp.tile([P, QS], F32R, name=f"qpadB{i}") for i in range(1)]
            for i in range(1):
                nc.vector.tensor_copy(qpadA[i][:], zsc[:])
                nc.vector.tensor_copy(qpadB[i][:], zsc[:])

            QT_g = kvp.tile([P, 2, NQ], F32R)
            KT_g = kvp.tile([P, 2, S], F32R)
            # per (toktile, pair, head j): [V_head(64) | 1 | 0(63)]
            V_gp = kvp.tile([P, ST, 2, 2, P], F32R)
            for t in range(ST):
                nc.vector.tensor_copy(
                    V_gp[:, t], zsc[:].rearrange("p (a b m) -> p a b m", a=2, b=2))
            one_r = qpp.tile([P, 1], F32R)
            nc.vector.tensor_copy(one_r[:], ones_f[:, 0:1])
            for t in range(ST):
                for pi in range(2):
                    for j in range(2):
                        nc.vector.tensor_copy(V_gp[:, t, pi, j, 64:65], one_r[:])
            it_count = 0

            for g in range(NG):
                for pl in range(2):   # head pairs 2g, 2g+1
                    pr = 2 * g + pl
                    wq_t = wsp.tile([P, KD // 2, 2, P], F8, tag="wq_t")
                    nc.sync.dma_start(wq_t[:], Wq[pr])
                    for q in range(NQS):
                        ps = aps.tile([P, QS], F32, tag="pp", bufs=2)
                        for kp in range(KD // 2):
                            nc.tensor.matmul(
                                ps[:], wq_t[:, kp, :, :],
                                xn_kvT[:, 2 * kp:2 * kp + 2, q * QS:(q + 1) * QS],
                                start=(kp == 0), stop=(kp == KD // 2 - 1),
                                perf_mode=PM.DoubleRow, skip_group_check=True)
                        nc.vector.tensor_scalar_add(
                            QT_g[:, pl, q * QS:(q + 1) * QS], ps[:], bq_t[:, pr:pr + 1])
                    wk_t = wsp.tile([P, KD // 2, 2, P], F8, tag="wk_t")
                    nc.sync.dma_start(wk_t[:], Wk[pr])
                    for q in range(NKS):
                        ps = aps.tile([P, QS], F32, tag="pp", bufs=2)
                        for kp in range(KD // 2):
                            nc.tensor.matmul(
                                ps[:], wk_t[:, kp, :, :],
                                xn_kvT[:, 2 * kp:2 * kp + 2, q * QS:(q + 1) * QS],
                                start=(kp == 0), stop=(kp == KD // 2 - 1),
                                perf_mode=PM.DoubleRow, skip_group_check=True)
                        nc.vector.tensor_scalar_add(
                            KT_g[:, pl, q * QS:(q + 1) * QS], ps[:], bk_t[:, pr:pr + 1])
                wv_t = wsp.tile([P, KD // 2, 2, 256], F8, tag="wv_t", bufs=1)
                nc.sync.dma_start(wv_t[:], Wv[g])
                for t in range(ST):
                    ps = aps.tile([P, QS], F32, tag="pp", bufs=2)
                    for kp in range(KD // 2):
                        nc.tensor.matmul(
                            ps[:, 0:256],
                            xn_kvT[:, 2 * kp:2 * kp + 2, t * P:(t + 1) * P],
                            wv_t[:, kp, :, :],
                            start=(kp == 0), stop=(kp == KD // 2 - 1),
                            perf_mode=PM.DoubleRow, skip_group_check=True)
                    for pi in range(2):
                        nc.vector.tensor_tensor(
                            V_gp[:, t, pi, :, 0:64],
                            ps[:, pi * 128:(pi + 1) * 128].rearrange("p (j m) -> p j m", j=2),
                            bv_rep[:, g * 256 + pi * 128:g * 256 + (pi + 1) * 128].rearrange(
                                "p (j m) -> p j m", j=2), ALU.add)

                if DEBUG and g == 0:
                    nc.sync.dma_start(dbg["kt0"], KT_g[:])
                    nc.sync.dma_start(dbg["v0"], V_gp[:])
                    nc.sync.dma_start(dbg["qt0"], QT_g[:])

                for q in range(NQS):
                    for pl in range(2):
                        pr = 2 * g + pl
                        i = it_count % 1
                        it_count += 1
                        qsl = slice(q * QS, (q + 1) * QS)
                        nc.vector.tensor_copy(qpadA[i][0:64, :], QT_g[0:64, pl, qsl])
                        nc.vector.tensor_copy(qpadB[i][64:128, :], QT_g[64:128, pl, qsl])
                        av1 = aps.tile([P, QS], F32, tag="av1")
                        av2 = aps.tile([P, QS], F32, tag="av2")
                        for kt in range(ST):
                            ktsl = slice(kt * P, (kt + 1) * P)
                            sAB = aps.tile([P, 2, QS], F32, tag="sAB", bufs=2)
                            nc.tensor.matmul(sAB[:, 0, :], KT_g[:, pl, ktsl], qpadA[i][:],
                                             start=True, stop=True)
                            nc.tensor.matmul(sAB[:, 1, :], KT_g[:, pl, ktsl], qpadB[i][:],
                                             start=True, stop=True)
                            eAB = expp.tile([P, 2, QS], F32R, tag="eAB")
                            nc.scalar.activation(eAB[:], sAB[:], AF.Exp, scale=0.125 / (WS * WS))
                            eA = eAB[:, 0, :]
                            eB = eAB[:, 1, :]
                            if DEBUG and g == 0 and q == 0 and pl == 0 and kt == 0:
                                nc.sync.dma_start(dbg["e0"], eA)
                                s0c = rcp.tile([P, QS], F32, tag="s0c")
                                nc.vector.tensor_copy(s0c[:], sAB[:, 0, :])
                                nc.sync.dma_start(dbg["s0"], s0c[:])
                            st, sp = (kt == 0), (kt == ST - 1)
                            nc.tensor.matmul(av1[:], V_gp[:, kt, pl, 0, :], eA,
                                             start=st, stop=sp, skip_group_check=True)
                            nc.tensor.matmul(av2[:], V_gp[:, kt, pl, 1, :], eB,
                                             start=st, stop=sp, skip_group_check=True)
                        # free the av psums fast: copy to SBUF, divide from there
                        avc = rcp.tile([65, 2, QS], F32, tag="avc")
                        nc.vector.tensor_copy(avc[0:65, 0, :], av1[0:65, :])
                        nc.vector.tensor_copy(avc[0:65, 1, :], av2[0:65, :])
                        nc.vector.reciprocal(avc[64:65, 0, :], avc[64:65, 0, :])
                        nc.vector.reciprocal(avc[64:65, 1, :], avc[64:65, 1, :])
                        rcd = dramp.tile([2, QS], F32, tag="rcd", bufs=2)
                        nc.sync.dma_start(rcd[0:1, :], avc[64:65, 0, :])
                        nc.sync.dma_start(rcd[1:2, :], avc[64:65, 1, :])
                        bcA = rcp.tile([64, QS], F32, tag="bcA")
                        bcB = rcp.tile([64, QS], F32, tag="bcB")

                        def _b64(row_ap):
                            return bass.AP(tensor=row_ap.tensor, offset=row_ap.offset,
                                           ap=[[0, 64]] + list(row_ap.ap)[1:])

                        nc.sync.dma_start(bcA[:], _b64(rcd[0:1, :]))
                        nc.sync.dma_start(bcB[:], _b64(rcd[1:2, :]))
                        if DEBUG and g == 0 and q == 0 and pl == 0:
                            nc.sync.dma_start(dbg["av0"], avc[:])
                        nc.vector.scalar_tensor_tensor(
                            RT_h[q][0:64, pr, :], avc[0:64, 0, :], 1.0 / WS,
                            bcA[:], ALU.mult, ALU.mult)
                        stB = rcp.tile([64, QS], F8, tag="stB")
                        nc.vector.scalar_tensor_tensor(
                            stB[:], avc[0:64, 1, :], 1.0 / WS, bcB[:],
                            ALU.mult, ALU.mult)
                        nc.sync.dma_start(RT_h[q][64:128, pr, :], stB[:])

        xn_es.close()

        if DEBUG:
            for h in range(NQS):
                nc.sync.dma_start(
                    dbg["rt"].rearrange("p k (h w) -> p k h w", h=NQS)[:, :, h, :], RT_h[h][:])

        # ---- Phase 4a: O-projection + residual -> x2 (DRAM) ----
        with tc.tile_pool(name="p4tmp", bufs=2) as p4t, \
             tc.tile_pool(name="p4ps", bufs=2, space="PSUM") as ps4, \
             tc.tile_pool(name="p4tps", bufs=6, space="PSUM") as ps4t:
            for q in range(NQS):
                attnT = p4t.tile([P, KD, QS], F32R, tag="attnT")
                for mt in range(KD):
                    wo_t = p4t.tile([P, KD // 2, 2, P], F8, tag="wo_t")
                    nc.sync.dma_start(wo_t[:], Wo[mt])
                    ps = ps4.tile([P, QS], F32, tag="pp")
                    for kp in range(KD // 2):
                        nc.tensor.matmul(
                            ps[:], wo_t[:, kp, :, :],
                            RT_h[q][:, 2 * kp:2 * kp + 2, :],
                            start=(kp == 0), stop=(kp == KD // 2 - 1),
                            perf_mode=PM.DoubleRow, skip_group_check=True)
                    nc.vector.tensor_scalar(
                        attnT[:, mt, :], ps[:], scalar1=1.0 / WS,
                        scalar2=bo_t[:, mt:mt + 1], op0=ALU.mult, op1=ALU.add)
                for j in range(QS // P):
                    tt = q * (QS // P) + j
                    xr_t = p4t.tile([P, D], F32, tag="xr_t")
                    nc.sync.dma_start(xr_t[:], xkv[tt * P:(tt + 1) * P, :])
                    x2_t = x2d[:, tt, :]
                    for m2 in range(KD // 2):
                        pst = ps4t.tile([P, 2, P], F32, tag="tp")
                        for h in range(2):
                            nc.tensor.transpose(
                                pst[:, h, :].bitcast(F32R),
                                attnT[:, 2 * m2 + h, j * P:(j + 1) * P], ident[:])
                        nc.vector.tensor_tensor(
                            x2_t[:, 2 * m2 * P:(2 * m2 + 2) * P],
                            pst[:].rearrange("p a m -> p (a m)"),
                            xr_t[:, 2 * m2 * P:(2 * m2 + 2) * P], ALU.add)
                    if DEBUG:
                        nc.sync.dma_start(dbg["x2"][:, tt, :], x2d[:, tt, :])
        rt_es.close()

        # ---- Phase 4b: LN2 -> xn2T ----
        xn2_es = ExitStack()
        xn2p = xn2_es.enter_context(tc.tile_pool(name="xn2", bufs=1))
        xn2T_h = [xn2p.tile([P, KD, 2, QS], F8, name=f"xn2T{h}") for h in range(NQS)]
        with tc.tile_pool(name="p4btmp", bufs=4) as p4bt, \
             tc.tile_pool(name="p4bs", bufs=4) as p4bs, \
             tc.tile_pool(name="ln2", bufs=1) as ln2p, \
             tc.tile_pool(name="p4bps", bufs=6, space="PSUM") as ps4b:
            g2_rep = ln2p.tile([P, D], F32)
            nc.gpsimd.dma_start(g2_rep[:], bcast_ap(g2))
            be2_rep = ln2p.tile([P, D], F32)
            nc.gpsimd.dma_start(be2_rep[:], bcast_ap(be2))
            eps2_t = ln2p.tile([P, 1], F32)
            nc.vector.memset(eps2_t[:], EPS)

            for tt in range(QTT):
                x2_t = x2d[:, tt, :]
                stats = p4bs.tile([P, 2, 6], F32, tag="stats2")
                xv = x2_t.rearrange("p (s f) -> p s f", s=2)
                for s in range(2):
                    nc.vector.bn_stats(stats[:, s, :], xv[:, s, :])
                mv = p4bs.tile([P, 2], F32, tag="mv2")
                nc.vector.bn_aggr(mv[:], stats[:])
                std = p4bs.tile([P, 1], F32, tag="std2")
                nc.scalar.activation(std[:], mv[:, 1:2], AF.Sqrt, bias=eps2_t[:])
                nc.vector.reciprocal(std[:], std[:])
                xn2_t = p4bt.tile([P, D], F32R, tag="xn2_t")
                nc.vector.tensor_scalar(
                    xn2_t[:], x2_t, scalar1=mv[:, 0:1], scalar2=std[:],
                    op0=ALU.subtract, op1=ALU.mult)
                if ln_affine:
                    nc.vector.tensor_tensor(xn2_t[:], xn2_t[:], g2_rep[:], ALU.mult)
                    nc.vector.tensor_tensor(xn2_t[:], xn2_t[:], be2_rep[:], ALU.add)
                hs_i, loc = tt // (QS // P), (tt % (QS // P)) * P
                for j2 in range(KD // 2):
                    pst = ps4b.tile([P, 2, P], F32, tag="tp")
                    for h in range(2):
                        nc.tensor.transpose(
                            pst[:, h, :].bitcast(F32R),
                            xn2_t[:, (2 * j2 + h) * P:(2 * j2 + h + 1) * P], ident[:])
                    hi = xn2T_h[hs_i][:, 2 * j2:2 * j2 + 2, 0, loc:loc + P]
                    nc.vector.tensor_copy(hi, pst[:])
                    nc.vector.scalar_tensor_tensor(
                        xn2T_h[hs_i][:, 2 * j2:2 * j2 + 2, 1, loc:loc + P],
                        pst[:], 1.0, hi, ALU.mult, ALU.subtract)

        # ---- Phase 5: MLP (h1 in bf16, single full-width token pass) ----
        with tc.tile_pool(name="p5tmp", bufs=3) as p5t, \
             tc.tile_pool(name="h1", bufs=1) as h1p, \
             tc.tile_pool(name="w2st", bufs=2) as w2p, \
             tc.tile_pool(name="p5ps", bufs=2, space="PSUM") as ps5, \
             tc.tile_pool(name="p5tps", bufs=4, space="PSUM") as ps5t:
            n_hslice = NQS
            HW_ = NQ // n_hslice
            out_acc = h1p.tile([P, QTT, D], F32)
            for hs in range(n_hslice):
                h1T = h1p.tile([P, FT, 2, HW_], F8, tag="h1T")
                for ft in range(FT):
                    w1_t = p5t.tile([P, KD, 2, P], F8, tag="w1_t")
                    nc.sync.dma_start(w1_t[:], W1[ft])
                    for sl in range(HW_ // QS):
                        gsl = (hs * HW_ + sl * QS) // QS
                        ps = ps5.tile([P, QS], F32, tag="pp")
                        for kp in range(KD // 2):
                            nc.tensor.matmul(
                                ps[:], w1_t[:, 2 * kp:2 * kp + 2, 1, :],
                                xn2T_h[gsl][:, 2 * kp:2 * kp + 2, 0, :],
                                start=(kp == 0), stop=False,
                                perf_mode=PM.DoubleRow, skip_group_check=True)
                        for k in range(KD):
                            nc.tensor.matmul(
                                ps[:], w1_t[:, k, :, :], xn2T_h[gsl][:, k, :, :],
                                start=False, stop=(k == KD - 1),
                                perf_mode=PM.DoubleRow, skip_group_check=True)
                        h1b = p5t.tile([P, QS], BF16, tag="h1b", bufs=2)
                        nc.scalar.activation(h1b[:], ps[:], AF.Gelu,
                                             bias=b1_t[:, ft:ft + 1], scale=1.0 / WS)
                        hi = h1T[:, ft, 0, sl * QS:(sl + 1) * QS]
                        nc.gpsimd.tensor_copy(hi, h1b[:])
                        nc.vector.scalar_tensor_tensor(
                            h1T[:, ft, 1, sl * QS:(sl + 1) * QS], h1b[:],
                            1.0, hi, ALU.mult, ALU.subtract)
                for mt in range(KD):
                    w2_t = w2p.tile([P, FT, 2, P], F8, tag="w2_t")
                    nc.sync.dma_start(w2_t[:], W2[mt])
                    for sl in range(HW_ // QS):
                        ssl_loc = slice(sl * QS, (sl + 1) * QS)
                        ps = ps5.tile([P, QS], F32, tag="pp")
                        for fp in range(FT // 2):
                            nc.tensor.matmul(
                                ps[:], w2_t[:, 2 * fp:2 * fp + 2, 1, :],
                                h1T[:, 2 * fp:2 * fp + 2, 0, ssl_loc],
                                start=(fp == 0), stop=False,
                                perf_mode=PM.DoubleRow, skip_group_check=True)
                        for k in range(FT):
                            nc.tensor.matmul(
                                ps[:], w2_t[:, k, :, :], h1T[:, k, :, ssl_loc],
                                start=False, stop=(k == FT - 1),
                                perf_mode=PM.DoubleRow, skip_group_check=True)
                        outT = p5t.tile([P, QS], F32R, tag="outT", bufs=2)
                        nc.vector.tensor_scalar(
                            outT[:], ps[:], scalar1=1.0 / WS2,
                            scalar2=b2_t[:, mt:mt + 1], op0=ALU.mult, op1=ALU.add)
                        for j in range(QS // P):
                            tt = hs * (HW_ // P) + sl * (QS // P) + j
                            pst = ps5t.tile([P, P], F32, tag="tp")
                            nc.tensor.transpose(pst[:].bitcast(F32R),
                                                outT[:, j * P:(j + 1) * P], ident[:])
                            nc.vector.tensor_copy(out_acc[:, tt, mt * P:(mt + 1) * P], pst[:])
            for tt in range(QTT):
                x2_t = x2d[:, tt, :]
                ob = p5t.tile([P, D], F32, tag="ob")
                nc.vector.tensor_tensor(ob[:], out_acc[:, tt, :], x2_t, ALU.add)
                nc.sync.dma_start(out[tt * P:(tt + 1) * P, :], ob[:])

        xn2_es.close()
        es.close()

    nc.compile()
    return nc


def kernel(**inputs):
    inputs = {k: np.ascontiguousarray(np.asarray(v), dtype=np.float32)
              for k, v in inputs.items()}
    ln_affine = not (
        np.all(inputs["ln1_g"] == 1.0) and np.all(inputs["ln1_b"] == 0.0)
        and np.all(inputs["ln2_g"] == 1.0) and np.all(inputs["ln2_b"] == 0.0))
    key = ("nc", ln_affine, MLP_BF16)
    if key not in _CACHE:
        _CACHE[key] = _build(ln_affine=ln_affine, mlp_bf16=MLP_BF16)
    nc = _CACHE[key]

    x = inputs["x"]
    f8 = __import__("ml_dtypes").float8_e4m3

    def attn_w(W, mw, sc):
        # [Din, Dout] -> [Dout/mw, 128, KD/2, 2, mw] scaled fp8 (DoubleRow lhsT)
        Dout = W.shape[1]
        t = (W * sc).astype(np.float32).reshape(
            KD // 2, 2, P, Dout // mw, mw).transpose(3, 2, 0, 1, 4)
        return np.ascontiguousarray(t).astype(f8)

    def mlp_w(W, sc, nk):
        # [Din, Dout] -> [Dout/128, 128, nk, 2, 128]; slot0=lo, slot1=hi
        Din, Dout = W.shape
        Ws = (W * sc).astype(np.float32)
        hi = Ws.astype(f8)
        lo = (Ws - hi.astype(np.float32)).astype(f8)

        def tile4(A):
            return A.reshape(nk, P, Dout // P, P).transpose(2, 1, 0, 3)

        outw = np.empty((Dout // P, P, nk, 2, P), dtype=f8)
        outw[:, :, :, 0, :] = tile4(lo)
        outw[:, :, :, 1, :] = tile4(hi)
        return np.ascontiguousarray(outw)

    shared = {
        "Wq": attn_w(inputs["Wq"], P, WS), "Wk": attn_w(inputs["Wk"], P, WS),
        "Wv": attn_w(inputs["Wv"], 256, WS), "Wo": attn_w(inputs["Wo"], P, WS),
        "W1": mlp_w(inputs["W1"], WS, KD),
        "W2": mlp_w(inputs["W2"], WS2, FT),
        "bq": inputs["bq"] * np.float32(WS), "bk": inputs["bk"] * np.float32(WS),
        "bv": inputs["bv"] * np.float32(WS), "bo": inputs["bo"],
        "b1": inputs["b1"], "b2": inputs["b2"],
        "g1": inputs["ln1_g"], "be1": inputs["ln1_b"],
        "g2": inputs["ln2_g"], "be2": inputs["ln2_b"],
    }
    in_maps = []
    for c in range(8):
        b, half = c // 2, c % 2
        m = dict(shared)
        # query half first; attention is permutation-invariant over kv order
        m["xkv"] = np.ascontiguousarray(
            np.concatenate([x[b, half * NQ:(half + 1) * NQ, :],
                            x[b, (1 - half) * NQ:(2 - half) * NQ, :]], axis=0))
        in_maps.append(m)

    trace = bool(int(os.environ.get("KERNEL_TRACE", "0")))
    kw = {}
    if trace:
        kw = dict(trace=True, tmpdir=os.environ.get("KERNEL_TRACE_DIR") or None)
    res = bass_utils.run_bass_kernel_spmd(nc, in_maps, core_ids=list(range(8)), **kw)
    _CACHE["last_results"] = res
    _CACHE["nc"] = nc
    _CACHE["last_in_maps"] = in_maps

    outa = np.empty((B, S, D), dtype=np.float32)
    for c in range(8):
        b, half = c // 2, c % 2
        outa[b, half * NQ:(half + 1) * NQ, :] = res.results[c]["out"]
    return outa



# revision 40
# speedup vs baseline: 1.0812x; 1.0417x over previous
"""Trainium2 Bass kernel for a dense transformer encoder block (B=4, S=2048,
D=1024, H=16, MLP=4096) — fp8-e4m3 DoubleRow hybrid.

Sharding: 8 cores = 4 batch elements x 2 query-halves, no collectives. Each
core's kv sequence is host-reordered so its 1024 query tokens come first
(attention is permutation-invariant over keys); K/V are computed for the full
2048-token sequence.

Per-core dataflow is feature-major ("T" = [feature, token]); all matmuls have
contraction 128 (f32r) or 256 (fp8 DoubleRow) on partitions:
  LN1 (token-major, bn_stats) -> PE-transpose -> xnT stored fp8    [phase 1]
  Q/K/V projections: fp8 DoubleRow (weights host-scaled x32, fp8;  [phase 3]
    each instruction contracts 2 k-tiles at 0.5 cycles/row). Q/K/V psums stay
    32x-scaled in f32r; the 32x32 factor folds into the softmax exp scale.
  scores/softmax/AV unchanged from the f32r formulation: zero-padded 2-head
    packing (K=128), exp on ACT with scale 0.125/1024, fused AV+denominator
    via an augmented [V | 1 | 0] lhsT, reciprocal + DRAM-roundtrip broadcast;
    the RT multiply descales V's 32x and emits fp8 for the O-projection.
  O-projection: fp8 DoubleRow (Wo x32, descale at the psum copy)   [phase 4a]
  LN2 -> xn2T stored as fp8 hi/lo split (hi + residual)            [phase 4b]
  MLP: 3-chain error-compensated fp8 DoubleRow                     [phase 5]
    W@x ~= Wh@xh + Wh@xl + Wl@xh with hi/lo fp8 splits of both operands;
    the two cross terms ride single DoubleRow instructions whose two k-slots
    hold (Wlo, xhi) and (Whi, xlo). W1 scaled x32, W2 x64 (clears e4m3's
    subnormal floor so the lo residues survive); descales fold into the
    gelu scale and the existing psum->SBUF copies. gelu output goes through
    bf16, then hi/lo fp8 for fc2.

Numerics validated in numpy and on hardware: end-to-end relmax ~3.7e-3
(budget 2e-2). Weights are host-retiled to [tile, 128, k/2, 2, m] fp8 so each
DoubleRow lhsT slice is a contiguous block.

Cost-model (TimelineSim) span: ~662 us/core (baseline f32r kernel: ~862).
x2 (attention output + residual) stays SBUF-resident (the fp8 tensors
freed ~100KB/partition), eliminating the baseline's x2 DRAM roundtrip.
PE work drops from ~1.50M to ~1.08M cycles: QKV/O projections 393K->98K and
fc1/fc2 524K->393K (3-chain), with scores/AV (524K) left in f32r — converting
those needs 4-head score packing + fp8 softmax weights, which in turn needs
psum/ACT restructuring that the in-order engine queues punish (measured).
"""

import os
import sys

sys.path.insert(0, "/opt/trn_rl_repo")

from contextlib import ExitStack

import numpy as np

import concourse.bass as bass
import concourse.tile as tile
from concourse import bacc, bass_utils, mybir
from concourse.masks import make_identity

F32 = mybir.dt.float32
F32R = mybir.dt.float32r
BF16 = mybir.dt.bfloat16
F8 = mybir.dt.float8e4
PM = mybir.MatmulPerfMode
WS = 32.0
WS2 = 64.0
AF = mybir.ActivationFunctionType
ALU = mybir.AluOpType

B, S, D = 4, 2048, 1024
H, DH, MLP = 16, 64, 4096
P = 128
KD = D // P            # 8 partition tiles over D
FT = MLP // P          # 32 partition tiles over MLP dim
NQ = S // 2            # 1024 query tokens per core
ST = S // P            # 16 kv token tiles
QTT = NQ // P          # 8 q token tiles
QS = 512               # free-dim slice
NQS = NQ // QS         # 2
NKS = S // QS          # 4
NG = 4                 # head groups
EPS = 1e-6
DEBUG = bool(int(os.environ.get("KERNEL_DEBUG", "0")))
MLP_BF16 = bool(int(os.environ.get("KERNEL_MLP_BF16", "0")))

_CACHE = {}


def _build(ln_affine=True, mlp_bf16=True):
    nc = bacc.Bacc(None, target_bir_lowering=False, debug=False, num_devices=8)

    xkv = nc.dram_tensor("xkv", [S, D], F32, kind="ExternalInput").ap()
    # weights arrive host-tiled: [tile, p, kd, m] so each SBUF weight tile is
    # one contiguous DRAM block (4KB+ per-partition DMA chunks)
    KP = KD // 2
    Wq = nc.dram_tensor("Wq", [KD, P, KP, 2, P], F8, kind="ExternalInput").ap()
    Wk = nc.dram_tensor("Wk", [KD, P, KP, 2, P], F8, kind="ExternalInput").ap()
    Wv = nc.dram_tensor("Wv", [NG, P, KP, 2, 256], F8, kind="ExternalInput").ap()
    Wo = nc.dram_tensor("Wo", [KD, P, KP, 2, P], F8, kind="ExternalInput").ap()
    W1 = nc.dram_tensor("W1", [FT, P, KD, 2, P], F8, kind="ExternalInput").ap()
    W2 = nc.dram_tensor("W2", [KD, P, FT, 2, P], F8, kind="ExternalInput").ap()
    bq = nc.dram_tensor("bq", [D], F32, kind="ExternalInput").ap()
    bk = nc.dram_tensor("bk", [D], F32, kind="ExternalInput").ap()
    bv = nc.dram_tensor("bv", [D], F32, kind="ExternalInput").ap()
    bo = nc.dram_tensor("bo", [D], F32, kind="ExternalInput").ap()
    b1 = nc.dram_tensor("b1", [MLP], F32, kind="ExternalInput").ap()
    b2 = nc.dram_tensor("b2", [D], F32, kind="ExternalInput").ap()
    g1 = nc.dram_tensor("g1", [D], F32, kind="ExternalInput").ap()
    be1 = nc.dram_tensor("be1", [D], F32, kind="ExternalInput").ap()
    g2 = nc.dram_tensor("g2", [D], F32, kind="ExternalInput").ap()
    be2 = nc.dram_tensor("be2", [D], F32, kind="ExternalInput").ap()
    out = nc.dram_tensor("out", [NQ, D], F32, kind="ExternalOutput").ap()

    dbg = {}
    if DEBUG:
        dbg["xnkvT"] = nc.dram_tensor("d_xnkvT", [P, KD, S], F8, kind="ExternalOutput").ap()
        dbg["qt0"] = nc.dram_tensor("d_qt0", [P, 2, NQ], F32R, kind="ExternalOutput").ap()
        dbg["kt0"] = nc.dram_tensor("d_kt0", [P, 2, S], F32R, kind="ExternalOutput").ap()
        dbg["v0"] = nc.dram_tensor("d_v0", [P, ST, 2, 2, P], F32R, kind="ExternalOutput").ap()
        dbg["rt"] = nc.dram_tensor("d_rt", [P, KD, NQ], F8, kind="ExternalOutput").ap()
        dbg["e0"] = nc.dram_tensor("d_e0", [P, QS], F32R, kind="ExternalOutput").ap()
        dbg["s0"] = nc.dram_tensor("d_s0", [P, QS], F32, kind="ExternalOutput").ap()
        dbg["av0"] = nc.dram_tensor("d_av0", [65, 2, QS], F32, kind="ExternalOutput").ap()
        dbg["x2"] = nc.dram_tensor("d_x2", [P, QTT, D], F32, kind="ExternalOutput").ap()

    def bcast_ap(vec):
        # [D] dram vector -> [128, D] partition-replicated DMA source
        return bass.AP(tensor=vec.tensor, offset=vec.offset, ap=[[0, P]] + list(vec.ap))



    with tile.TileContext(nc) as tc:
        es = ExitStack()
        params = es.enter_context(tc.tile_pool(name="params", bufs=1))
        dramp = es.enter_context(tc.tile_pool(name="dram", bufs=1, space="DRAM"))
        x2sb = es.enter_context(tc.tile_pool(name="x2sb", bufs=1))
        x2d = x2sb.tile([P, QTT, D], F32)

        ident_f = params.tile([P, P], F32)
        make_identity(nc, ident_f)
        ident = params.tile([P, P], F32R)
        nc.vector.tensor_copy(ident[:], ident_f[:])
        ones_f = params.tile([P, 1], F32)
        nc.vector.memset(ones_f[:, 0:1], 1.0)
        zb_t = params.tile([P, 1], F32)
        nc.vector.memset(zb_t[:], 0.0)

        def pvec(v, n, nm):  # [n*128] -> [128, n] (dim o*128+p -> [p, o])
            t = params.tile([P, n], F32, name=nm)
            nc.sync.dma_start(t[:], v.rearrange("(o p) -> p o", p=P))
            return t

        bq_t = pvec(bq, KD, "bq_t")
        bk_t = pvec(bk, KD, "bk_t")
        bo_t = pvec(bo, KD, "bo_t")
        b2_t = pvec(b2, KD, "b2_t")
        b1_t = pvec(b1, FT, "b1_t")
        bv_rep = params.tile([P, D], F32)
        nc.gpsimd.dma_start(bv_rep[:], bcast_ap(bv))

        rt_es = ExitStack()
        rtp = rt_es.enter_context(tc.tile_pool(name="rt", bufs=1))
        RT_h = [rtp.tile([P, KD, QS], F8, name=f"RT{h}") for h in range(NQS)]

        xn_es = ExitStack()
        xnp = xn_es.enter_context(tc.tile_pool(name="xn", bufs=1))
        xn_kvT = xnp.tile([P, KD, S], F8)

        # ---- Phase 1: LN1 + transpose to feature-major ----
        with tc.tile_pool(name="p1tmp", bufs=4) as p1t, \
             tc.tile_pool(name="p1s", bufs=4) as p1s, \
             tc.tile_pool(name="ln1", bufs=1) as ln1p, \
             tc.tile_pool(name="p1ps", bufs=6, space="PSUM") as ps1:
            g1_rep = ln1p.tile([P, D], F32)
            nc.gpsimd.dma_start(g1_rep[:], bcast_ap(g1))
            be1_rep = ln1p.tile([P, D], F32)
            nc.gpsimd.dma_start(be1_rep[:], bcast_ap(be1))
            eps_t = ln1p.tile([P, 1], F32)
            nc.vector.memset(eps_t[:], EPS)

            for t in range(ST):
                x_t = p1t.tile([P, D], F32, tag="x_t")
                nc.sync.dma_start(x_t[:], xkv[t * P:(t + 1) * P, :])
                stats = p1s.tile([P, 2, 6], F32, tag="stats")
                xv = x_t[:].rearrange("p (s f) -> p s f", s=2)
                for s in range(2):
                    nc.vector.bn_stats(stats[:, s, :], xv[:, s, :])
                mv = p1s.tile([P, 2], F32, tag="mv")
                nc.vector.bn_aggr(mv[:], stats[:])
                std = p1s.tile([P, 1], F32, tag="std")
                nc.scalar.activation(std[:], mv[:, 1:2], AF.Sqrt, bias=eps_t[:])
                nc.vector.reciprocal(std[:], std[:])
                negms = p1s.tile([P, 1], F32, tag="negms")
                nc.vector.scalar_tensor_tensor(
                    negms[:], mv[:, 0:1], -1.0, std[:], ALU.mult, ALU.mult)
                xn_t = p1t.tile([P, D], F32R, tag="xn_t")
                nc.scalar.activation(
                    xn_t[:], x_t[:], AF.Identity, scale=std[:], bias=negms[:])
                if ln_affine:
                    nc.vector.tensor_tensor(xn_t[:], xn_t[:], g1_rep[:], ALU.mult)
                    nc.vector.tensor_tensor(xn_t[:], xn_t[:], be1_rep[:], ALU.add)
                for j2 in range(KD // 2):
                    pst = ps1.tile([P, 2, P], F32, tag="tp")
                    for h in range(2):
                        nc.tensor.transpose(
                            pst[:, h, :].bitcast(F32R),
                            xn_t[:, (2 * j2 + h) * P:(2 * j2 + h + 1) * P], ident[:])
                    if j2 % 2 == 0:
                        nc.vector.tensor_copy(
                            xn_kvT[:, 2 * j2:2 * j2 + 2, t * P:(t + 1) * P], pst[:])
                    else:
                        nc.scalar.activation(
                            xn_kvT[:, 2 * j2:2 * j2 + 2, t * P:(t + 1) * P],
                            pst[:], AF.Identity, bias=zb_t[:])

        if DEBUG:
            nc.sync.dma_start(dbg["xnkvT"], xn_kvT[:])

        # ---- Phase 3: per-group QKV projection + attention ----
        with tc.tile_pool(name="kv", bufs=1) as kvp, \
             tc.tile_pool(name="wst", bufs=2) as wsp, \
             tc.tile_pool(name="expp", bufs=2) as expp, \
             tc.tile_pool(name="qpad", bufs=1) as qpp, \
             tc.tile_pool(name="rcbc", bufs=1) as rcp, \
             tc.tile_pool(name="aps", bufs=1, space="PSUM") as aps:

            zsc = qpp.tile([P, QS], F32)
            nc.vector.memset(zsc[:], 0.0)
            qpadA = [qpp.tile([P, QS], F32R, name=f"qpadA{i}") for i in range(1)]
            qpadB = [qpp.tile([P, QS], F32R, name=f"qpadB{i}") for i in range(1)]
            for i in range(1):
                nc.vector.tensor_copy(qpadA[i][:], zsc[:])
                nc.vector.tensor_copy(qpadB[i][:], zsc[:])

            QT_g = kvp.tile([P, 2, NQ], F32R)
            KT_g = kvp.tile([P, 2, S], F32R)
            # per (toktile, pair, head j): [V_head(64) | 1 | 0(63)]
            V_gp = kvp.tile([P, ST, 2, 2, P], F32R)
            for t in range(ST):
                nc.vector.tensor_copy(
                    V_gp[:, t], zsc[:].rearrange("p (a b m) -> p a b m", a=2, b=2))
            one_r = qpp.tile([P, 1], F32R)
            nc.vector.tensor_copy(one_r[:], ones_f[:, 0:1])
            for t in range(ST):
                for pi in range(2):
                    for j in range(2):
                        nc.vector.tensor_copy(V_gp[:, t, pi, j, 64:65], one_r[:])
            it_count = 0

            for g in range(NG):
                for pl in range(2):   # head pairs 2g, 2g+1
                    pr = 2 * g + pl
                    wq_t = wsp.tile([P, KD // 2, 2, P], F8, tag="wq_t")
                    nc.sync.dma_start(wq_t[:], Wq[pr])
                    for q in range(NQS):
                        ps = aps.tile([P, QS], F32, tag="pp", bufs=2)
                        for kp in range(KD // 2):
                            nc.tensor.matmul(
                                ps[:], wq_t[:, kp, :, :],
                                xn_kvT[:, 2 * kp:2 * kp + 2, q * QS:(q + 1) * QS],
                                start=(kp == 0), stop=(kp == KD // 2 - 1),
                                perf_mode=PM.DoubleRow, skip_group_check=True)
                        nc.vector.tensor_scalar_add(
                            QT_g[:, pl, q * QS:(q + 1) * QS], ps[:], bq_t[:, pr:pr + 1])
                    wk_t = wsp.tile([P, KD // 2, 2, P], F8, tag="wk_t")
                    nc.sync.dma_start(wk_t[:], Wk[pr])
                    for q in range(NKS):
                        ps = aps.tile([P, QS], F32, tag="pp", bufs=2)
                        for kp in range(KD // 2):
                            nc.tensor.matmul(
                                ps[:], wk_t[:, kp, :, :],
                                xn_kvT[:, 2 * kp:2 * kp + 2, q * QS:(q + 1) * QS],
                                start=(kp == 0), stop=(kp == KD // 2 - 1),
                                perf_mode=PM.DoubleRow, skip_group_check=True)
                        nc.vector.tensor_scalar_add(
                            KT_g[:, pl, q * QS:(q + 1) * QS], ps[:], bk_t[:, pr:pr + 1])
                wv_t = wsp.tile([P, KD // 2, 2, 256], F8, tag="wv_t", bufs=1)
                nc.sync.dma_start(wv_t[:], Wv[g])
                for t in range(ST):
                    ps = aps.tile([P, QS], F32, tag="pp", bufs=2)
                    for kp in range(KD // 2):
                        nc.tensor.matmul(
                            ps[:, 0:256],
                            xn_kvT[:, 2 * kp:2 * kp + 2, t * P:(t + 1) * P],
                            wv_t[:, kp, :, :],
                            start=(kp == 0), stop=(kp == KD // 2 - 1),
                            perf_mode=PM.DoubleRow, skip_group_check=True)
                    for pi in range(2):
                        nc.vector.tensor_tensor(
                            V_gp[:, t, pi, :, 0:64],
                            ps[:, pi * 128:(pi + 1) * 128].rearrange("p (j m) -> p j m", j=2),
                            bv_rep[:, g * 256 + pi * 128:g * 256 + (pi + 1) * 128].rearrange(
                                "p (j m) -> p j m", j=2), ALU.add)

                if DEBUG and g == 0:
                    nc.sync.dma_start(dbg["kt0"], KT_g[:])
                    nc.sync.dma_start(dbg["v0"], V_gp[:])
                    nc.sync.dma_start(dbg["qt0"], QT_g[:])

                for q in range(NQS):
                    for pl in range(2):
                        pr = 2 * g + pl
                        i = it_count % 1
                        it_count += 1
                        qsl = slice(q * QS, (q + 1) * QS)
                        nc.vector.tensor_copy(qpadA[i][0:64, :], QT_g[0:64, pl, qsl])
                        nc.vector.tensor_copy(qpadB[i][64:128, :], QT_g[64:128, pl, qsl])
                        av1 = aps.tile([P, QS], F32, tag="av1")
                        av2 = aps.tile([P, QS], F32, tag="av2")
                        for kt in range(ST):
                            ktsl = slice(kt * P, (kt + 1) * P)
                            sAB = aps.tile([P, 2, QS], F32, tag="sAB", bufs=2)
                            nc.tensor.matmul(sAB[:, 0, :], KT_g[:, pl, ktsl], qpadA[i][:],
                                             start=True, stop=True)
                            nc.tensor.matmul(sAB[:, 1, :], KT_g[:, pl, ktsl], qpadB[i][:],
                                             start=True, stop=True)
                            eAB = expp.tile([P, 2, QS], F32R, tag="eAB")
                            nc.scalar.activation(eAB[:], sAB[:], AF.Exp, scale=0.125 / (WS * WS))
                            eA = eAB[:, 0, :]
                            eB = eAB[:, 1, :]
                            if DEBUG and g == 0 and q == 0 and pl == 0 and kt == 0:
                                nc.sync.dma_start(dbg["e0"], eA)
                                s0c = rcp.tile([P, QS], F32, tag="s0c")
                                nc.vector.tensor_copy(s0c[:], sAB[:, 0, :])
                                nc.sync.dma_start(dbg["s0"], s0c[:])
                            st, sp = (kt == 0), (kt == ST - 1)
                            nc.tensor.matmul(av1[:], V_gp[:, kt, pl, 0, :], eA,
                                             start=st, stop=sp, skip_group_check=True)
                            nc.tensor.matmul(av2[:], V_gp[:, kt, pl, 1, :], eB,
                                             start=st, stop=sp, skip_group_check=True)
                        # free the av psums fast: copy to SBUF, divide from there
                        avc = rcp.tile([65, 2, QS], F32, tag="avc")
                        nc.vector.tensor_copy(avc[0:65, 0, :], av1[0:65, :])
                        nc.vector.tensor_copy(avc[0:65, 1, :], av2[0:65, :])
                        nc.vector.reciprocal(avc[64:65, 0, :], avc[64:65, 0, :])
                        nc.vector.reciprocal(avc[64:65, 1, :], avc[64:65, 1, :])
                        rcd = dramp.tile([2, QS], F32, tag="rcd", bufs=2)
                        nc.sync.dma_start(rcd[0:1, :], avc[64:65, 0, :])
                        nc.sync.dma_start(rcd[1:2, :], avc[64:65, 1, :])
                        bcA = rcp.tile([64, QS], F32, tag="bcA")
                        bcB = rcp.tile([64, QS], F32, tag="bcB")

                        def _b64(row_ap):
                            return bass.AP(tensor=row_ap.tensor, offset=row_ap.offset,
                                           ap=[[0, 64]] + list(row_ap.ap)[1:])

                        nc.sync.dma_start(bcA[:], _b64(rcd[0:1, :]))
                        nc.sync.dma_start(bcB[:], _b64(rcd[1:2, :]))
                        if DEBUG and g == 0 and q == 0 and pl == 0:
                            nc.sync.dma_start(dbg["av0"], avc[:])
                        nc.vector.scalar_tensor_tensor(
                            RT_h[q][0:64, pr, :], avc[0:64, 0, :], 1.0 / WS,
                            bcA[:], ALU.mult, ALU.mult)
                        stB = rcp.tile([64, QS], F8, tag="stB")
                        nc.vector.scalar_tensor_tensor(
                            stB[:], avc[0:64, 1, :], 1.0 / WS, bcB[:],
                            ALU.mult, ALU.mult)
                        nc.sync.dma_start(RT_h[q][64:128, pr, :], stB[:])

        xn_es.close()

        if DEBUG:
            for h in range(NQS):
                nc.sync.dma_start(
                    dbg["rt"].rearrange("p k (h w) -> p k h w", h=NQS)[:, :, h, :], RT_h[h][:])

        # ---- Phase 4a: O-projection + residual -> x2 (DRAM) ----
        with tc.tile_pool(name="p4tmp", bufs=2) as p4t, \
             tc.tile_pool(name="p4ps", bufs=2, space="PSUM") as ps4, \
             tc.tile_pool(name="p4tps", bufs=6, space="PSUM") as ps4t:
            for q in range(NQS):
                attnT = p4t.tile([P, KD, QS], F32R, tag="attnT")
                for mt in range(KD):
                    wo_t = p4t.tile([P, KD // 2, 2, P], F8, tag="wo_t")
                    nc.sync.dma_start(wo_t[:], Wo[mt])
                    ps = ps4.tile([P, QS], F32, tag="pp")
                    for kp in range(KD // 2):
                        nc.tensor.matmul(
                            ps[:], wo_t[:, kp, :, :],
                            RT_h[q][:, 2 * kp:2 * kp + 2, :],
                            start=(kp == 0), stop=(kp == KD // 2 - 1),
                            perf_mode=PM.DoubleRow, skip_group_check=True)
                    nc.vector.tensor_scalar(
                        attnT[:, mt, :], ps[:], scalar1=1.0 / WS,
                        scalar2=bo_t[:, mt:mt + 1], op0=ALU.mult, op1=ALU.add)
                for j in range(QS // P):
                    tt = q * (QS // P) + j
                    xr_t = p4t.tile([P, D], F32, tag="xr_t")
                    nc.sync.dma_start(xr_t[:], xkv[tt * P:(tt + 1) * P, :])
                    x2_t = x2d[:, tt, :]
                    for m2 in range(KD // 2):
                        pst = ps4t.tile([P, 2, P], F32, tag="tp")
                        for h in range(2):
                            nc.tensor.transpose(
                                pst[:, h, :].bitcast(F32R),
                                attnT[:, 2 * m2 + h, j * P:(j + 1) * P], ident[:])
                        nc.vector.tensor_tensor(
                            x2_t[:, 2 * m2 * P:(2 * m2 + 2) * P],
                            pst[:].rearrange("p a m -> p (a m)"),
                            xr_t[:, 2 * m2 * P:(2 * m2 + 2) * P], ALU.add)
                    if DEBUG:
                        nc.sync.dma_start(dbg["x2"][:, tt, :], x2d[:, tt, :])
        rt_es.close()

        # ---- Phase 4b: LN2 -> xn2T ----
        xn2_es = ExitStack()
        xn2p = xn2_es.enter_context(tc.tile_pool(name="xn2", bufs=1))
        xn2T_h = [xn2p.tile([P, KD, 2, QS], F8, name=f"xn2T{h}") for h in range(NQS)]
        with tc.tile_pool(name="p4btmp", bufs=4) as p4bt, \
             tc.tile_pool(name="p4bs", bufs=4) as p4bs, \
             tc.tile_pool(name="ln2", bufs=1) as ln2p, \
             tc.tile_pool(name="p4bps", bufs=6, space="PSUM") as ps4b:
            g2_rep = ln2p.tile([P, D], F32)
            nc.gpsimd.dma_start(g2_rep[:], bcast_ap(g2))
            be2_rep = ln2p.tile([P, D], F32)
            nc.gpsimd.dma_start(be2_rep[:], bcast_ap(be2))
            eps2_t = ln2p.tile([P, 1], F32)
            nc.vector.memset(eps2_t[:], EPS)

            for tt in range(QTT):
                x2_t = x2d[:, tt, :]
                stats = p4bs.tile([P, 2, 6], F32, tag="stats2")
                xv = x2_t.rearrange("p (s f) -> p s f", s=2)
                for s in range(2):
                    nc.vector.bn_stats(stats[:, s, :], xv[:, s, :])
                mv = p4bs.tile([P, 2], F32, tag="mv2")
                nc.vector.bn_aggr(mv[:], stats[:])
                std = p4bs.tile([P, 1], F32, tag="std2")
                nc.scalar.activation(std[:], mv[:, 1:2], AF.Sqrt, bias=eps2_t[:])
                nc.vector.reciprocal(std[:], std[:])
                negms2 = p4bs.tile([P, 1], F32, tag="negms2")
                nc.vector.scalar_tensor_tensor(
                    negms2[:], mv[:, 0:1], -1.0, std[:], ALU.mult, ALU.mult)
                xn2_t = p4bt.tile([P, D], F32R, tag="xn2_t")
                nc.scalar.activation(
                    xn2_t[:], x2_t, AF.Identity, scale=std[:], bias=negms2[:])
                if ln_affine:
                    nc.vector.tensor_tensor(xn2_t[:], xn2_t[:], g2_rep[:], ALU.mult)
                    nc.vector.tensor_tensor(xn2_t[:], xn2_t[:], be2_rep[:], ALU.add)
                hs_i, loc = tt // (QS // P), (tt % (QS // P)) * P
                for j2 in range(KD // 2):
                    pst = ps4b.tile([P, 2, P], F32, tag="tp")
                    for h in range(2):
                        nc.tensor.transpose(
                            pst[:, h, :].bitcast(F32R),
                            xn2_t[:, (2 * j2 + h) * P:(2 * j2 + h + 1) * P], ident[:])
                    hi = xn2T_h[hs_i][:, 2 * j2:2 * j2 + 2, 0, loc:loc + P]
                    nc.scalar.activation(hi, pst[:], AF.Identity, bias=zb_t[:])
                    nc.vector.scalar_tensor_tensor(
                        xn2T_h[hs_i][:, 2 * j2:2 * j2 + 2, 1, loc:loc + P],
                        pst[:], 1.0, hi, ALU.mult, ALU.subtract)

        # ---- Phase 5: MLP (h1 in bf16, single full-width token pass) ----
        with tc.tile_pool(name="p5tmp", bufs=3) as p5t, \
             tc.tile_pool(name="h1", bufs=1) as h1p, \
             tc.tile_pool(name="w2st", bufs=2) as w2p, \
             tc.tile_pool(name="p5ps", bufs=2, space="PSUM") as ps5, \
             tc.tile_pool(name="p5tps", bufs=4, space="PSUM") as ps5t:
            n_hslice = NQS
            HW_ = NQ // n_hslice
            out_acc = h1p.tile([P, QTT, D], F32)
            for hs in range(n_hslice):
                h1T = h1p.tile([P, FT, 2, HW_], F8, tag="h1T")
                for ft in range(FT):
                    w1_t = p5t.tile([P, KD, 2, P], F8, tag="w1_t")
                    nc.sync.dma_start(w1_t[:], W1[ft])
                    for sl in range(HW_ // QS):
                        gsl = (hs * HW_ + sl * QS) // QS
                        ps = ps5.tile([P, QS], F32, tag="pp")
                        for kp in range(KD // 2):
                            nc.tensor.matmul(
                                ps[:], w1_t[:, 2 * kp:2 * kp + 2, 1, :],
                                xn2T_h[gsl][:, 2 * kp:2 * kp + 2, 0, :],
                                start=(kp == 0), stop=False,
                                perf_mode=PM.DoubleRow, skip_group_check=True)
                        for k in range(KD):
                            nc.tensor.matmul(
                                ps[:], w1_t[:, k, :, :], xn2T_h[gsl][:, k, :, :],
                                start=False, stop=(k == KD - 1),
                                perf_mode=PM.DoubleRow, skip_group_check=True)
                        h1b = p5t.tile([P, QS], BF16, tag="h1b", bufs=2)
                        nc.scalar.activation(h1b[:], ps[:], AF.Gelu,
                                             bias=b1_t[:, ft:ft + 1], scale=1.0 / WS)
                        hi = h1T[:, ft, 0, sl * QS:(sl + 1) * QS]
                        nc.gpsimd.tensor_copy(hi, h1b[:])
                        nc.vector.scalar_tensor_tensor(
                            h1T[:, ft, 1, sl * QS:(sl + 1) * QS], h1b[:],
                            1.0, hi, ALU.mult, ALU.subtract)
                for mt in range(KD):
                    w2_t = w2p.tile([P, FT, 2, P], F8, tag="w2_t")
                    nc.sync.dma_start(w2_t[:], W2[mt])
                    for sl in range(HW_ // QS):
                        ssl_loc = slice(sl * QS, (sl + 1) * QS)
                        ps = ps5.tile([P, QS], F32, tag="pp")
                        for fp in range(FT // 2):
                            nc.tensor.matmul(
                                ps[:], w2_t[:, 2 * fp:2 * fp + 2, 1, :],
                                h1T[:, 2 * fp:2 * fp + 2, 0, ssl_loc],
                                start=(fp == 0), stop=False,
                                perf_mode=PM.DoubleRow, skip_group_check=True)
                        for k in range(FT):
                            nc.tensor.matmul(
                                ps[:], w2_t[:, k, :, :], h1T[:, k, :, ssl_loc],
                                start=False, stop=(k == FT - 1),
                                perf_mode=PM.DoubleRow, skip_group_check=True)
                        outT = p5t.tile([P, QS], F32R, tag="outT", bufs=2)
                        nc.vector.tensor_scalar(
                            outT[:], ps[:], scalar1=1.0 / WS2,
                            scalar2=b2_t[:, mt:mt + 1], op0=ALU.mult, op1=ALU.add)
                        for j in range(QS // P):
                            tt = hs * (HW_ // P) + sl * (QS // P) + j
                            pst = ps5t.tile([P, P], F32, tag="tp")
                            nc.tensor.transpose(pst[:].bitcast(F32R),
                                                outT[:, j * P:(j + 1) * P], ident[:])
                            nc.vector.tensor_copy(out_acc[:, tt, mt * P:(mt + 1) * P], pst[:])
            for tt in range(QTT):
                x2_t = x2d[:, tt, :]
                ob = p5t.tile([P, D], F32, tag="ob")
                nc.vector.tensor_tensor(ob[:], out_acc[:, tt, :], x2_t, ALU.add)
                nc.sync.dma_start(out[tt * P:(tt + 1) * P, :], ob[:])

        xn2_es.close()
        es.close()

    nc.compile()
    return nc


def kernel(**inputs):
    inputs = {k: np.ascontiguousarray(np.asarray(v), dtype=np.float32)
              for k, v in inputs.items()}
    ln_affine = not (
        np.all(inputs["ln1_g"] == 1.0) and np.all(inputs["ln1_b"] == 0.0)
        and np.all(inputs["ln2_g"] == 1.0) and np.all(inputs["ln2_b"] == 0.0))
    key = ("nc", ln_affine, MLP_BF16)
    if key not in _CACHE:
        _CACHE[key] = _build(ln_affine=ln_affine, mlp_bf16=MLP_BF16)
    nc = _CACHE[key]

    x = inputs["x"]
    f8 = __import__("ml_dtypes").float8_e4m3

    def attn_w(W, mw, sc):
        # [Din, Dout] -> [Dout/mw, 128, KD/2, 2, mw] scaled fp8 (DoubleRow lhsT)
        Dout = W.shape[1]
        t = (W * sc).astype(np.float32).reshape(
            KD // 2, 2, P, Dout // mw, mw).transpose(3, 2, 0, 1, 4)
        return np.ascontiguousarray(t).astype(f8)

    def mlp_w(W, sc, nk):
        # [Din, Dout] -> [Dout/128, 128, nk, 2, 128]; slot0=lo, slot1=hi
        Din, Dout = W.shape
        Ws = (W * sc).astype(np.float32)
        hi = Ws.astype(f8)
        lo = (Ws - hi.astype(np.float32)).astype(f8)

        def tile4(A):
            return A.reshape(nk, P, Dout // P, P).transpose(2, 1, 0, 3)

        outw = np.empty((Dout // P, P, nk, 2, P), dtype=f8)
        outw[:, :, :, 0, :] = tile4(lo)
        outw[:, :, :, 1, :] = tile4(hi)
        return np.ascontiguousarray(outw)

    shared = {
        "Wq": attn_w(inputs["Wq"], P, WS), "Wk": attn_w(inputs["Wk"], P, WS),
        "Wv": attn_w(inputs["Wv"], 256, WS), "Wo": attn_w(inputs["Wo"], P, WS),
        "W1": mlp_w(inputs["W1"], WS, KD),
        "W2": mlp_w(inputs["W2"], WS2, FT),
        "bq": inputs["bq"] * np.float32(WS), "bk": inputs["bk"] * np.float32(WS),
        "bv": inputs["bv"] * np.float32(WS), "bo": inputs["bo"],
        "b1": inputs["b1"], "b2": inputs["b2"],
        "g1": inputs["ln1_g"], "be1": inputs["ln1_b"],
        "g2": inputs["ln2_g"], "be2": inputs["ln2_b"],
    }
    in_maps = []
    for c in range(8):
        b, half = c // 2, c % 2
        m = dict(shared)
        # query half first; attention is permutation-invariant over kv order
        m["xkv"] = np.ascontiguousarray(
            np.concatenate([x[b, half * NQ:(half + 1) * NQ, :],
                            x[b, (1 - half) * NQ:(2 - half) * NQ, :]], axis=0))
        in_maps.append(m)

    trace = bool(int(os.environ.get("KERNEL_TRACE", "0")))
    kw = {}
    if trace:
        kw = dict(trace=True, tmpdir=os.environ.get("KERNEL_TRACE_DIR") or None)
    res = bass_utils.run_bass_kernel_spmd(nc, in_maps, core_ids=list(range(8)), **kw)
    _CACHE["last_results"] = res
    _CACHE["nc"] = nc
    _CACHE["last_in_maps"] = in_maps

    outa = np.empty((B, S, D), dtype=np.float32)
    for c in range(8):
        b, half = c // 2, c % 2
        outa[b, half * NQ:(half + 1) * NQ, :] = res.results[c]["out"]
    return outa

